# revision 39
# baseline (speedup 1.0000x reference)
"""BiLSTM-CRF Trainium2 kernel — 8-core TIME-chunked scan.

Contract: kernel(**inputs) takes the FULL unsharded inputs (numpy arrays,
keyed as in reference.setup_inputs()) and returns the FULL [B, T, TAGS, TAGS]
crf_scores array.

Sharding: the 512-step scan is latency-bound (per-step serial chain of
~2us across PE->ACT->VEC->ACT->VEC), and per-instruction costs are almost
entirely fixed overhead, so batching all 64 sequences into one op costs
barely more than 8.  We therefore split TIME, not batch: core c computes
time chunk [64c, 64c+64) for the full batch, running its fwd/bwd scans W
extra warmup steps from zero state.  The LSTM here is strongly contractive
(forget gate ~0.5), so the warmup truncation error decays ~0.5^W.

Cell math is reformulated tanh-only (sigma(x) = (tanh(x/2)+1)/2, with
weights pre-scaled on host, hidden state carried as h' = 2h and cell
state as cc = 2c):
    T = tanh(z')            z' blocks [o, i, f, g] with f,i,o halved
    P = [(T_i+1)*T_g | (T_f+1)*cc]      (one scalar_tensor_tensor)
    cc_new = 0.5*P_f + P_i              (one scalar_tensor_tensor,
                                         written into next step's Q tile)
    TC = tanh(0.5*cc_new)               (ACT free scale)
    h' = (T_o+1)*TC                     (one scalar_tensor_tensor)
This drops one VEC op + one ACT op vs the plain sigmoid chain and keeps
the per-step serial path at PE -> ACT -> VEC -> VEC -> ACT -> VEC.
"""
import sys
import types
from contextlib import ExitStack

import ml_dtypes
import numpy as np

import concourse.bacc as bacc
import concourse.bass as bass
import concourse.mybir as mybir
import concourse.tile as tile
from concourse import library_config
from concourse.bass_utils import run_bass_kernel_spmd

# ---- problem dims (hardcoded per spec) ----
VOCAB = 30000
VPAD = 30720      # emb table padded; row 30000 is all-zeros (warmup token)
ZTOK = 30000
EMB = 256
HD = 128          # per-direction hidden
G4 = 512          # 4*HD gates
TAGS = 16
B, T = 64, 512
NCORES = 8

CH = 64           # time chunk per core
W = 16            # warmup steps
S = CH + W        # scan steps per direction
NTS = CH + 2 * W  # timesteps of tokens/zin per core (union fwd+bwd windows)
NTOK = NTS * B    # tokens per core
TPB = 8           # timesteps per 512-token block
NB = NTS // TPB   # gather/gemm blocks
WB = W // TPB     # warmup blocks per end

BF16 = mybir.dt.bfloat16
F32 = mybir.dt.float32
I16 = mybir.dt.int16
AF = mybir.ActivationFunctionType
ALU = mybir.AluOpType

# gate order in reference (jnp.split): i, f, g, o.  Device block order is
# [o, i, f, g] so that W-pair = [T_i|T_f] and V-pair = [T_g|c] are both
# contiguous ascending (c lives in block 4 of the Q tile).
_PERM = np.concatenate([
    np.arange(384, 512),   # o
    np.arange(0, 128),     # i
    np.arange(128, 256),   # f
    np.arange(256, 384),   # g
])
# per-block extra scale for the tanh-half trick: f,i,o rows halved; g not.
_BLK_SCALE = np.concatenate([
    np.full(128, 0.5),     # o
    np.full(128, 0.5),     # i
    np.full(128, 0.5),     # f
    np.full(128, 1.0),     # g
])


def _ensure_ntff_hook():
    """The RL image's antenv lacks axon_hooks; inject it so trace=True works."""
    if "antenv.axon_hooks" in sys.modules:
        return
    mod = types.ModuleType("antenv.axon_hooks")
    mod._hook = None
    mod.set_axon_ntff_profile_hook = lambda h: setattr(mod, "_hook", h)
    mod.get_axon_ntff_profile_hook = lambda: mod._hook
    sys.modules["antenv.axon_hooks"] = mod
    try:
        import antenv
        antenv.axon_hooks = mod
        from trn_agent_boot.trn_boot import _ntff_profile_via_ctypes
        mod.set_axon_ntff_profile_hook(
            _ntff_profile_via_ctypes("/opt/axon/libaxon_pjrt.so"))
    except Exception:
        pass


def build(steps: int = S, nts: int = NTS):
    """Build + compile the per-core Bass program."""
    nb = nts // TPB
    ntok = nts * B
    ch = steps - W
    nc = bacc.Bacc("TRN2", target_bir_lowering=False, debug=False)

    # ---- DRAM I/O ----
    emb_d = nc.dram_tensor("emb", [VPAD, EMB], BF16, kind="ExternalInput")
    idx_d = nc.dram_tensor("idx", [128, ntok // 16], I16, kind="ExternalInput")
    wihT_d = {d: nc.dram_tensor(f"wihT_{d}", [EMB, G4], BF16, kind="ExternalInput")
              for d in "fb"}
    whhT_d = {d: nc.dram_tensor(f"whhT_{d}", [HD, G4], BF16, kind="ExternalInput")
              for d in "fb"}
    # main bias + warmup-window bias (zeroed on edge cores), [128, 4] f32
    bias_d = {d: nc.dram_tensor(f"bias_{d}", [128, 4], F32, kind="ExternalInput")
              for d in "fb"}
    biasw_d = {d: nc.dram_tensor(f"biasw_{d}", [128, 4], F32, kind="ExternalInput")
               for d in "fb"}
    woutT_d = nc.dram_tensor("woutT", [2, HD, TAGS], BF16, kind="ExternalInput")
    trans_d = nc.dram_tensor("trans", [128, TAGS * TAGS], F32, kind="ExternalInput")
    ident_d = nc.dram_tensor("ident", [128, 128], BF16, kind="ExternalInput")
    crf_d = nc.dram_tensor("crf", [ch * B, TAGS * TAGS], F32, kind="ExternalOutput")

    with tile.TileContext(nc) as tc, ExitStack() as ctx:
        nc.gpsimd.load_library(library_config.mlp)
        const = ctx.enter_context(tc.tile_pool(name="const", bufs=1))
        big = ctx.enter_context(tc.tile_pool(name="big", bufs=1))
        # emission pools opened early so their PSUM banks / SBUF slots are
        # disjoint from the scan pools (no pool-release serialization).
        epsum = ctx.enter_context(tc.tile_pool(name="epsum", bufs=2, space="PSUM"))
        ecrf = ctx.enter_context(tc.tile_pool(name="ecrf", bufs=4))

        # ---- persistent SBUF ----
        idx_sb = const.tile([128, ntok // 16], I16)
        wihT = {d: const.tile([128, 2, G4], BF16, tag=f"wihT{d}", name=f"wihT{d}") for d in "fb"}
        whhT = {d: const.tile([HD, G4], BF16, tag=f"whhT{d}", name=f"whhT{d}") for d in "fb"}
        bias = {d: const.tile([128, 4], F32, tag=f"bias{d}", name=f"bias{d}") for d in "fb"}
        biasw = {d: const.tile([128, 4], F32, tag=f"biasw{d}", name=f"biasw{d}") for d in "fb"}
        woutT = const.tile([HD, 2, TAGS], BF16)
        trans = const.tile([128, TAGS * TAGS], F32)
        ident = const.tile([128, 128], BF16)

        # token embeddings, transposed: [128 emb-part, block, emb-half, 512 tok]
        xT = big.tile([128, nb, 2, 512], BF16, tag="xT")
        # quarter-granular staging for the fast-path first blocks (gather
        # out APs must be contiguous, so quarters get their own slots)
        xq = big.tile([128, 16, 2, 128], BF16, tag="xq")
        # input projections, [128 gate-part, t, block(oifg), batch]
        zin = {d: big.tile([128, nts, 4, B], BF16, tag=f"zin{d}", name=f"zin{d}")
               for d in "fb"}
        # h' histories (bf16), split into 16-col segments so the emission
        # epilogue's tile-granular deps bind to a segment (overlaps the
        # scan) instead of the whole history.
        # fwd: col k+1 = h' after fwd step k; real time t0+i at col W+1+i.
        # bwd: step j writes col steps-j (descending); real time t0+i at
        #   col i+1; col steps+1 is the zero init.
        # Segments: [0..W], then 16-col segments covering the real cols.
        hbnd = [0, W + 1] + [W + 1 + 16 * i for i in range(1, ch // 16 + 1)] \
            + [steps + 2]
        hseg = {d: [big.tile([128, hbnd[i + 1] - hbnd[i], B], BF16,
                             tag=f"h{d}{i}", name=f"h{d}{i}")
                    for i in range(len(hbnd) - 1)] for d in "fb"}

        def hcol(d, col):
            for i in range(len(hbnd) - 1):
                if col < hbnd[i + 1]:
                    return hseg[d][i][:, col - hbnd[i], :]
            raise AssertionError(col)

        def hspan2(d, c0):
            """[128, 2, B] span over cols c0, c0+1 (same segment)."""
            for i in range(len(hbnd) - 1):
                if c0 < hbnd[i + 1]:
                    assert c0 + 2 <= hbnd[i + 1], (d, c0)
                    return hseg[d][i][:, c0 - hbnd[i]:c0 - hbnd[i] + 2, :]
            raise AssertionError(c0)

        # ---- load constants / inputs ----
        nc.sync.dma_start(idx_sb[:], idx_d[:])
        for d in "fb":
            nc.sync.dma_start(wihT[d][:], wihT_d[d].rearrange("(k p) g -> p k g", p=128))
            nc.sync.dma_start(whhT[d][:], whhT_d[d][:])
            nc.sync.dma_start(bias[d][:], bias_d[d][:])
            nc.sync.dma_start(biasw[d][:], biasw_d[d][:])
        nc.sync.dma_start(woutT[:], woutT_d.rearrange("c h t -> h c t"))
        nc.sync.dma_start(trans[:], trans_d[:])
        nc.sync.dma_start(ident[:], ident_d[:])
        nc.gpsimd.memset(hcol("f", 0), 0.0)
        nc.gpsimd.memset(hcol("b", steps + 1), 0.0)

        # ---- embedding gather straight into x.T layout ----
        # fwd consumes blocks ascending from 0, bwd descending from nb-1.
        # The first two blocks of each direction are gathered in 128-token
        # quarters, interleaved by first-use time, so the scan can start
        # ~17us earlier (it only needs f:(0,q0) and b:(nb-1,q3) up front).
        def gather_q(qi, b_, q):
            nc.gpsimd.dma_gather(
                xq[:, qi, :, :],
                emb_d[:, :],
                idx_sb[:, 32 * b_ + 8 * q:32 * b_ + 8 * (q + 1)],
                128, 128, EMB,
                transpose=True,
            )

        quarters = []
        for j in range(8):
            quarters.append(("f", j // 4, j % 4))          # blocks 0,1 fwd
            quarters.append(("b", nb - 1 - j // 4, 3 - j % 4))  # nb-1, nb-2
        border = []
        for k in range(2, (nb + 1) // 2):
            border.append(k)
            if nb - 1 - k != k:
                border.append(nb - 1 - k)

        # ---- input projections: zin = x @ Wih.T + b ----
        # fwd needs blocks [0, nb-1-WB), bwd needs [WB, nb).  Warmup-window
        # blocks (first WB for fwd, last WB for bwd) use the biasw tensor.
        zpsum = ctx.enter_context(tc.tile_pool(name="zpsum", bufs=2, space="PSUM"))
        _flip = [0]

        def zin_unit(d, b_, c):
            """One (dir, block, gate-slice) GEMM + bias copyback."""
            warm = (d == "f" and b_ < WB) or (d == "b" and b_ >= nb - WB)
            bsel = biasw[d] if warm else bias[d]
            zp = zpsum.tile([128, 512], F32, tag="zp")
            nc.tensor.matmul(
                zp[:], wihT[d][:, 0, 128 * c:128 * (c + 1)],
                xT[:, b_, 0, :],
                start=True, stop=False)
            nc.tensor.matmul(
                zp[:], wihT[d][:, 1, 128 * c:128 * (c + 1)],
                xT[:, b_, 1, :],
                start=False, stop=True)
            # strided copyback into [t, c, b] layout, bias folded in;
            # alternate ACT/VEC to balance engine load.
            dst = zin[d][:, TPB * b_:TPB * (b_ + 1), c, :]
            if _flip[0] % 3 == 0:
                nc.scalar.activation(dst, zp[:], AF.Identity,
                                     bias=bsel[:, c:c + 1])
            else:
                nc.vector.tensor_scalar(dst, zp[:], bsel[:, c:c + 1],
                                        None, ALU.add)
            _flip[0] += 1

        def zin_unit_q(qi, d, b_, c, q):
            """Quarter-block (128-token) GEMM + bias copyback."""
            warm = (d == "f" and b_ < WB) or (d == "b" and b_ >= nb - WB)
            bsel = biasw[d] if warm else bias[d]
            zq = zpsum.tile([128, 128], F32, tag="zp")
            nc.tensor.matmul(
                zq[:], wihT[d][:, 0, 128 * c:128 * (c + 1)],
                xq[:, qi, 0, :],
                start=True, stop=False)
            nc.tensor.matmul(
                zq[:], wihT[d][:, 1, 128 * c:128 * (c + 1)],
                xq[:, qi, 1, :],
                start=False, stop=True)
            dst = zin[d][:, TPB * b_ + 2 * q:TPB * b_ + 2 * q + 2, c, :]
            if _flip[0] % 3 == 0:
                nc.scalar.activation(dst, zq[:], AF.Identity,
                                     bias=bsel[:, c:c + 1])
            else:
                nc.vector.tensor_scalar(dst, zq[:], bsel[:, c:c + 1],
                                        None, ALU.add)
            _flip[0] += 1

        # Up-front fast path: quarter-granular gathers + GEMMs for the
        # first two blocks of each direction, interleaved by need time.
        # The rest is emitted inside the scan loop (one unit per step) so
        # the zin GEMMs fill engine-idle slots instead of fighting the
        # chain at scan start.
        for qi, (d, b_, q) in enumerate(quarters):
            gather_q(qi, b_, q)
            for c in range(4):
                zin_unit_q(qi, d, b_, c, q)
        for b_ in border:
            nc.gpsimd.dma_gather(
                xT[:, b_, :, :],
                emb_d[:, :],
                idx_sb[:, 32 * b_:32 * (b_ + 1)],
                512, 512, EMB,
                transpose=True,
            )

        # Deferred-block zin units: upfront emission measured best — the
        # copyback intrusions concentrate in the first ~20 scan steps and
        # the rest of the scan runs at the clean 2213ns chain period.
        # (Spreading them through the loop locked a +437ns/step regime.)
        for j in range(2, nb - 2):
            for d, b_ in (("f", j), ("b", nb - 1 - j)):
                for c in range(4):
                    zin_unit(d, b_, c)

        def inloop_units(k):
            return []

        # ---- the recurrent scan (fwd + bwd interleaved) ----
        with tc.tile_pool(name="spsum", bufs=4, space="PSUM") as spsum, \
             tc.tile_pool(name="sQ", bufs=10) as sQ, \
             tc.tile_pool(name="sP", bufs=10) as sP, \
             tc.tile_pool(name="sT", bufs=8) as sT:

            def new_z(k):
                """Fresh psum tiles for step k with zin injected (identity mm).
                Emitted one step ahead so gate mms fire as soon as h lands."""
                zt = {}
                for d in "fb":
                    ti = k if d == "f" else nts - 1 - k
                    zt[d] = spsum.tile([128, 4, B], F32, tag="z", name=f"z{d}")
                    nc.tensor.matmul(zt[d][:], ident[:],
                                     zin[d][:, ti, :, :],
                                     start=True, stop=False)
                return zt

            q = {d: sQ.tile([128, 5, B], F32, tag="q", name=f"q{d}") for d in "fb"}
            for d in "fb":
                nc.gpsimd.memset(q[d][:, 4, :], 0.0)
            z = new_z(0)
            for k in range(steps):
                for d, rd_col in (("f", k), ("b", steps + 1 - k)):
                    for c in range(4):
                        nc.tensor.matmul(
                            z[d][:, c, :],
                            whhT[d][:, 128 * c:128 * (c + 1)],
                            hcol(d, rd_col),
                            start=False, stop=(c == 3))
                z_cur, z = z, (new_z(k + 1) if k + 1 < steps else None)
                qn = {d: sQ.tile([128, 5, B], F32, tag="q", name=f"q{d}")
                      for d in "fb"}
                # Stage-interleaved emission: both dirs' ops alternate at
                # each chain stage so the engine queues lock the two chains
                # half a period out of phase.
                wr_col = {"f": k + 1, "b": steps - k}
                P = {}
                for d in "fb":
                    # T = tanh(z') into blocks [o,i,f,g]; state cc = 2*c
                    # sits in block 4.
                    nc.scalar.activation(q[d][:, 0:4, :], z_cur[d][:],
                                         AF.Tanh)
                for d in "fb":
                    # P = [(T_i+1)*T_g | (T_f+1)*cc]
                    P[d] = sP.tile([128, 2, B], F32, tag="P", name="P")
                    nc.vector.scalar_tensor_tensor(
                        P[d][:], q[d][:, 1:3, :], 1.0, q[d][:, 3:5, :],
                        ALU.add, ALU.mult)
                for d in "fb":
                    # cc_new = 2*c_new = 0.5*P1 + P0, written straight into
                    # the next step's Q tile (no separate state-fix op).
                    nc.vector.scalar_tensor_tensor(
                        qn[d][:, 4, :], P[d][:, 1, :], 0.5, P[d][:, 0, :],
                        ALU.mult, ALU.add)
                TC = {}
                for d in "fb":
                    TC[d] = sT.tile([128, B], F32, tag="TC", name="TC")
                    nc.scalar.activation(TC[d][:], qn[d][:, 4, :], AF.Tanh,
                                         scale=0.5)
                for d in "fb":
                    # h' = (T_o + 1) * TC
                    nc.vector.scalar_tensor_tensor(
                        hcol(d, wr_col[d]), q[d][:, 0, :], 1.0, TC[d][:],
                        ALU.add, ALU.mult)
                q = qn
                for (ud, ub, uc) in inloop_units(k):
                    zin_unit(ud, ub, uc)

        # ---- emission + CRF broadcast-add + store ----
        # chunk n covers local times 2n, 2n+1 (128 tokens);
        # hf cols W+1+2n..W+2+2n, hb cols 2n+1..2n+2.
        nchunks = ch // 2
        order = sorted(range(nchunks),
                       key=lambda n: max(W + 2 + 2 * n, steps - 1 - 2 * n))
        if True:
            for n in order:
                e = epsum.tile([128, TAGS], F32, tag="e")
                nc.tensor.matmul(e[:], hspan2("f", W + 1 + 2 * n),
                                 woutT[:, 0, :], start=True, stop=False)
                nc.tensor.matmul(e[:], hspan2("b", 1 + 2 * n),
                                 woutT[:, 1, :], start=False, stop=True)
                crf_sb = ecrf.tile([128, TAGS * TAGS], F32, tag="crf")
                e_b = e[:, None, :].to_broadcast([128, TAGS, TAGS])
                nc.vector.tensor_tensor(crf_sb[:], e_b, trans[:], ALU.add)
                nc.sync.dma_start(crf_d[128 * n:128 * (n + 1), :], crf_sb[:])

    nc.compile()
    _assert_ldw_pairing(nc)
    return nc


def _assert_ldw_pairing(nc):
    """Every non-self-loading matmul must directly follow an InstLdweights
    whose weights AP matches the matmul's weights operand."""
    for f in nc.m.functions:
        for bb in f.blocks:
            prev_pe = None
            for ins in bb.instructions:
                if ins.engine != mybir.EngineType.PE:
                    continue
                if isinstance(ins, mybir.InstMatmult) and ins.ldweights is False:
                    assert isinstance(prev_pe, mybir.InstLdweights), (
                        f"{ins.name}: non-self-loading matmul not preceded by "
                        f"ldweights (got {type(prev_pe).__name__})")
                    assert repr(prev_pe.ins[0]) == repr(ins.ins[1]), (
                        f"{ins.name}: weights mismatch with {prev_pe.name}")
                prev_pe = ins


_CACHE = {}


def _get_nc():
    if "nc" not in _CACHE:
        _CACHE["nc"] = build()
    return _CACHE["nc"]


def _prep_dir(w_ih, w_hh, b):
    """Permute gates to [o,i,f,g]; apply tanh-half trick (f,i,o rows x0.5)
    and h'=2h compensation (all Whh x0.5)."""
    w_ih = np.asarray(w_ih, np.float32)[_PERM] * _BLK_SCALE[:, None]
    w_hh = np.asarray(w_hh, np.float32)[_PERM] * (0.5 * _BLK_SCALE[:, None])
    b = np.asarray(b, np.float32)[_PERM] * _BLK_SCALE
    wihT = np.ascontiguousarray(w_ih.T).astype(ml_dtypes.bfloat16)
    whhT = np.ascontiguousarray(w_hh.T).astype(ml_dtypes.bfloat16)
    bias = np.ascontiguousarray(b.reshape(4, 128).T).astype(np.float32)
    return wihT, whhT, bias


def make_in_maps(sentences, embedding, W_ih_f, W_hh_f, b_f, W_ih_b, W_hh_b,
                 b_b, W_out, b_out, transition):
    emb = np.zeros((VPAD, EMB), np.float32)
    emb[:VOCAB] = np.asarray(embedding, np.float32)
    emb = emb.astype(ml_dtypes.bfloat16)
    wihT_f, whhT_f, bias_f = _prep_dir(W_ih_f, W_hh_f, b_f)
    wihT_b, whhT_b, bias_b = _prep_dir(W_ih_b, W_hh_b, b_b)
    wo = np.asarray(W_out, np.float32) * 0.5   # h' = 2h compensation
    woutT = np.stack([np.ascontiguousarray(wo[:, :128].T),
                      np.ascontiguousarray(wo[:, 128:].T)])
    woutT = woutT.astype(ml_dtypes.bfloat16)  # [2, 128, 16]
    trans_aug = (np.asarray(transition, np.float32)
                 + np.asarray(b_out, np.float32)[None, :]).reshape(-1)  # [256]
    trans_rep = np.ascontiguousarray(
        np.broadcast_to(trans_aug, (128, 256))).astype(np.float32)
    ident = np.eye(128, dtype=ml_dtypes.bfloat16)
    zeros4 = np.zeros((128, 4), np.float32)

    # tokens per core: times [64c - W, 64c + 64 + W), batch-inner (t, b)
    # order; out-of-range times -> the zero embedding row (ZTOK).
    sent = np.asarray(sentences).astype(np.int64)  # [B, T]
    in_maps = []
    for c in range(NCORES):
        t_lo = CH * c - W
        times = np.arange(t_lo, t_lo + NTS)
        cols = np.clip(times, 0, T - 1)
        toks = sent[:, cols].T.copy()          # [NTS, B]
        toks[(times < 0) | (times >= T)] = ZTOK
        toks = toks.reshape(-1)                # (t, b) order, [NTOK]
        idx = np.tile(toks.reshape(NTOK // 16, 16).T.astype(np.int16), (8, 1))
        in_maps.append({
            "emb": emb, "idx": idx,
            "wihT_f": wihT_f, "wihT_b": wihT_b,
            "whhT_f": whhT_f, "whhT_b": whhT_b,
            "bias_f": bias_f, "bias_b": bias_b,
            "biasw_f": zeros4 if c == 0 else bias_f,
            "biasw_b": zeros4 if c == NCORES - 1 else bias_b,
            "woutT": woutT, "trans": trans_rep, "ident": ident,
        })
    return in_maps


def assemble_out(results):
    out = np.empty((B, T, TAGS, TAGS), np.float32)
    for c in range(NCORES):
        crf = results[c]["crf"].reshape(CH, B, TAGS, TAGS)
        out[:, CH * c:CH * (c + 1)] = crf.transpose(1, 0, 2, 3)
    return out


def kernel(**inputs):
    _ensure_ntff_hook()
    nc = _get_nc()
    in_maps = make_in_maps(**inputs)
    res = run_bass_kernel_spmd(nc, in_maps, list(range(NCORES)))
    return assemble_out(res.results)


# revision 40
# speedup vs baseline: 1.0031x; 1.0031x over previous
"""BiLSTM-CRF Trainium2 kernel — 8-core TIME-chunked scan.

Contract: kernel(**inputs) takes the FULL unsharded inputs (numpy arrays,
keyed as in reference.setup_inputs()) and returns the FULL [B, T, TAGS, TAGS]
crf_scores array.

Sharding: the 512-step scan is latency-bound (per-step serial chain of
~2us across PE->ACT->VEC->ACT->VEC), and per-instruction costs are almost
entirely fixed overhead, so batching all 64 sequences into one op costs
barely more than 8.  We therefore split TIME, not batch: core c computes
time chunk [64c, 64c+64) for the full batch, running its fwd/bwd scans W
extra warmup steps from zero state.  The LSTM here is strongly contractive
(forget gate ~0.5), so the warmup truncation error decays ~0.5^W.

Cell math is reformulated tanh-only (sigma(x) = (tanh(x/2)+1)/2, with
weights pre-scaled on host, hidden state carried as h' = 2h and cell
state as cc = 2c):
    T = tanh(z')            z' blocks [o, i, f, g] with f,i,o halved
    P = [(T_i+1)*T_g | (T_f+1)*cc]      (one scalar_tensor_tensor)
    cc_new = 0.5*P_f + P_i              (one scalar_tensor_tensor,
                                         written into next step's Q tile)
    TC = tanh(0.5*cc_new)               (ACT free scale)
    h' = (T_o+1)*TC                     (one scalar_tensor_tensor)
This drops one VEC op + one ACT op vs the plain sigmoid chain and keeps
the per-step serial path at PE -> ACT -> VEC -> VEC -> ACT -> VEC.
"""
import sys
import types
from contextlib import ExitStack

import ml_dtypes
import numpy as np

import concourse.bacc as bacc
import concourse.bass as bass
import concourse.mybir as mybir
import concourse.tile as tile
from concourse import library_config
from concourse.bass_utils import run_bass_kernel_spmd

# ---- problem dims (hardcoded per spec) ----
VOCAB = 30000
VPAD = 30720      # emb table padded; row 30000 is all-zeros (warmup token)
ZTOK = 30000
EMB = 256
HD = 128          # per-direction hidden
G4 = 512          # 4*HD gates
TAGS = 16
B, T = 64, 512
NCORES = 8

CH = 64           # time chunk per core
W = 16            # warmup steps
S = CH + W        # scan steps per direction
NTS = CH + 2 * W  # timesteps of tokens/zin per core (union fwd+bwd windows)
NTOK = NTS * B    # tokens per core
TPB = 8           # timesteps per 512-token block
NB = NTS // TPB   # gather/gemm blocks
WB = W // TPB     # warmup blocks per end

BF16 = mybir.dt.bfloat16
F32 = mybir.dt.float32
I16 = mybir.dt.int16
AF = mybir.ActivationFunctionType
ALU = mybir.AluOpType

# gate order in reference (jnp.split): i, f, g, o.  Device block order is
# [o, i, f, g] so that W-pair = [T_i|T_f] and V-pair = [T_g|c] are both
# contiguous ascending (c lives in block 4 of the Q tile).
_PERM = np.concatenate([
    np.arange(384, 512),   # o
    np.arange(0, 128),     # i
    np.arange(128, 256),   # f
    np.arange(256, 384),   # g
])
# per-block extra scale for the tanh-half trick: f,i,o rows halved; g not.
_BLK_SCALE = np.concatenate([
    np.full(128, 0.5),     # o
    np.full(128, 0.5),     # i
    np.full(128, 0.5),     # f
    np.full(128, 1.0),     # g
])


def _ensure_ntff_hook():
    """The RL image's antenv lacks axon_hooks; inject it so trace=True works."""
    if "antenv.axon_hooks" in sys.modules:
        return
    mod = types.ModuleType("antenv.axon_hooks")
    mod._hook = None
    mod.set_axon_ntff_profile_hook = lambda h: setattr(mod, "_hook", h)
    mod.get_axon_ntff_profile_hook = lambda: mod._hook
    sys.modules["antenv.axon_hooks"] = mod
    try:
        import antenv
        antenv.axon_hooks = mod
        from trn_agent_boot.trn_boot import _ntff_profile_via_ctypes
        mod.set_axon_ntff_profile_hook(
            _ntff_profile_via_ctypes("/opt/axon/libaxon_pjrt.so"))
    except Exception:
        pass


def build(steps: int = S, nts: int = NTS):
    """Build + compile the per-core Bass program."""
    nb = nts // TPB
    ntok = nts * B
    ch = steps - W
    nc = bacc.Bacc("TRN2", target_bir_lowering=False, debug=False)

    # ---- DRAM I/O ----
    emb_d = nc.dram_tensor("emb", [VPAD, EMB], BF16, kind="ExternalInput")
    idx_d = nc.dram_tensor("idx", [128, ntok // 16], I16, kind="ExternalInput")
    wihT_d = {d: nc.dram_tensor(f"wihT_{d}", [EMB, G4], BF16, kind="ExternalInput")
              for d in "fb"}
    whhT_d = {d: nc.dram_tensor(f"whhT_{d}", [HD, G4], BF16, kind="ExternalInput")
              for d in "fb"}
    # main bias + warmup-window bias (zeroed on edge cores), [128, 4] f32
    bias_d = {d: nc.dram_tensor(f"bias_{d}", [128, 4], F32, kind="ExternalInput")
              for d in "fb"}
    biasw_d = {d: nc.dram_tensor(f"biasw_{d}", [128, 4], F32, kind="ExternalInput")
               for d in "fb"}
    woutT_d = nc.dram_tensor("woutT", [2, HD, TAGS], BF16, kind="ExternalInput")
    trans_d = nc.dram_tensor("trans", [128, TAGS * TAGS], F32, kind="ExternalInput")
    ident_d = nc.dram_tensor("ident", [128, 128], BF16, kind="ExternalInput")
    crf_d = nc.dram_tensor("crf", [ch * B, TAGS * TAGS], F32, kind="ExternalOutput")

    with tile.TileContext(nc) as tc, ExitStack() as ctx:
        nc.gpsimd.load_library(library_config.mlp)
        const = ctx.enter_context(tc.tile_pool(name="const", bufs=1))
        big = ctx.enter_context(tc.tile_pool(name="big", bufs=1))
        # emission pools opened early so their PSUM banks / SBUF slots are
        # disjoint from the scan pools (no pool-release serialization).
        epsum = ctx.enter_context(tc.tile_pool(name="epsum", bufs=2, space="PSUM"))
        ecrf = ctx.enter_context(tc.tile_pool(name="ecrf", bufs=4))

        # ---- persistent SBUF ----
        idx_sb = const.tile([128, ntok // 16], I16)
        wihT = {d: const.tile([128, 2, G4], BF16, tag=f"wihT{d}", name=f"wihT{d}") for d in "fb"}
        whhT = {d: const.tile([HD, G4], BF16, tag=f"whhT{d}", name=f"whhT{d}") for d in "fb"}
        bias = {d: const.tile([128, 4], F32, tag=f"bias{d}", name=f"bias{d}") for d in "fb"}
        biasw = {d: const.tile([128, 4], F32, tag=f"biasw{d}", name=f"biasw{d}") for d in "fb"}
        woutT = const.tile([HD, 2, TAGS], BF16)
        trans = const.tile([128, TAGS * TAGS], F32)
        ident = const.tile([128, 128], BF16)

        # token embeddings, transposed: [128 emb-part, block, emb-half, 512 tok]
        xT = big.tile([128, nb, 2, 512], BF16, tag="xT")
        # quarter-granular staging for the fast-path first blocks (gather
        # out APs must be contiguous, so quarters get their own slots)
        xq = big.tile([128, 16, 2, 128], BF16, tag="xq")
        # input projections, [128 gate-part, t, block(oifg), batch]
        zin = {d: big.tile([128, nts, 4, B], BF16, tag=f"zin{d}", name=f"zin{d}")
               for d in "fb"}
        # h' histories (bf16), split into 16-col segments so the emission
        # epilogue's tile-granular deps bind to a segment (overlaps the
        # scan) instead of the whole history.
        # fwd: col k+1 = h' after fwd step k; real time t0+i at col W+1+i.
        # bwd: step j writes col steps-j (descending); real time t0+i at
        #   col i+1; col steps+1 is the zero init.
        # Segments: [0..W], then 16-col segments covering the real cols.
        hbnd = [0, W + 1] + [W + 1 + 16 * i for i in range(1, ch // 16 + 1)] \
            + [steps + 2]
        hseg = {d: [big.tile([128, hbnd[i + 1] - hbnd[i], B], BF16,
                             tag=f"h{d}{i}", name=f"h{d}{i}")
                    for i in range(len(hbnd) - 1)] for d in "fb"}

        def hcol(d, col):
            for i in range(len(hbnd) - 1):
                if col < hbnd[i + 1]:
                    return hseg[d][i][:, col - hbnd[i], :]
            raise AssertionError(col)

        def hspan2(d, c0):
            """[128, 2, B] span over cols c0, c0+1 (same segment)."""
            for i in range(len(hbnd) - 1):
                if c0 < hbnd[i + 1]:
                    assert c0 + 2 <= hbnd[i + 1], (d, c0)
                    return hseg[d][i][:, c0 - hbnd[i]:c0 - hbnd[i] + 2, :]
            raise AssertionError(c0)

        # ---- load constants / inputs ----
        nc.sync.dma_start(idx_sb[:], idx_d[:])
        for d in "fb":
            nc.sync.dma_start(wihT[d][:], wihT_d[d].rearrange("(k p) g -> p k g", p=128))
            nc.sync.dma_start(whhT[d][:], whhT_d[d][:])
            nc.sync.dma_start(bias[d][:], bias_d[d][:])
            nc.sync.dma_start(biasw[d][:], biasw_d[d][:])
        nc.sync.dma_start(woutT[:], woutT_d.rearrange("c h t -> h c t"))
        nc.sync.dma_start(trans[:], trans_d[:])
        nc.sync.dma_start(ident[:], ident_d[:])
        nc.gpsimd.memset(hcol("f", 0), 0.0)
        nc.gpsimd.memset(hcol("b", steps + 1), 0.0)

        # ---- embedding gather straight into x.T layout ----
        # fwd consumes blocks ascending from 0, bwd descending from nb-1.
        # The first two blocks of each direction are gathered in 128-token
        # quarters, interleaved by first-use time, so the scan can start
        # ~17us earlier (it only needs f:(0,q0) and b:(nb-1,q3) up front).
        def gather_q(qi, b_, q):
            nc.gpsimd.dma_gather(
                xq[:, qi, :, :],
                emb_d[:, :],
                idx_sb[:, 32 * b_ + 8 * q:32 * b_ + 8 * (q + 1)],
                128, 128, EMB,
                transpose=True,
            )

        quarters = []
        for j in range(8):
            quarters.append(("f", j // 4, j % 4))          # blocks 0,1 fwd
            quarters.append(("b", nb - 1 - j // 4, 3 - j % 4))  # nb-1, nb-2
        border = []
        for k in range(2, (nb + 1) // 2):
            border.append(k)
            if nb - 1 - k != k:
                border.append(nb - 1 - k)

        # ---- input projections: zin = x @ Wih.T + b ----
        # fwd needs blocks [0, nb-1-WB), bwd needs [WB, nb).  Warmup-window
        # blocks (first WB for fwd, last WB for bwd) use the biasw tensor.
        zpsum = ctx.enter_context(tc.tile_pool(name="zpsum", bufs=2, space="PSUM"))
        _flip = [0]

        def zin_unit(d, b_, c):
            """One (dir, block, gate-slice) GEMM + bias copyback."""
            warm = (d == "f" and b_ < WB) or (d == "b" and b_ >= nb - WB)
            bsel = biasw[d] if warm else bias[d]
            zp = zpsum.tile([128, 512], F32, tag="zp")
            nc.tensor.matmul(
                zp[:], wihT[d][:, 0, 128 * c:128 * (c + 1)],
                xT[:, b_, 0, :],
                start=True, stop=False)
            nc.tensor.matmul(
                zp[:], wihT[d][:, 1, 128 * c:128 * (c + 1)],
                xT[:, b_, 1, :],
                start=False, stop=True)
            # strided copyback into [t, c, b] layout, bias folded in;
            # alternate ACT/VEC to balance engine load.
            dst = zin[d][:, TPB * b_:TPB * (b_ + 1), c, :]
            if _flip[0] % 3 != 0:
                nc.scalar.activation(dst, zp[:], AF.Identity,
                                     bias=bsel[:, c:c + 1])
            else:
                nc.vector.tensor_scalar(dst, zp[:], bsel[:, c:c + 1],
                                        None, ALU.add)
            _flip[0] += 1

        def zin_unit_q(qi, d, b_, c, q):
            """Quarter-block (128-token) GEMM + bias copyback."""
            warm = (d == "f" and b_ < WB) or (d == "b" and b_ >= nb - WB)
            bsel = biasw[d] if warm else bias[d]
            zq = zpsum.tile([128, 128], F32, tag="zp")
            nc.tensor.matmul(
                zq[:], wihT[d][:, 0, 128 * c:128 * (c + 1)],
                xq[:, qi, 0, :],
                start=True, stop=False)
            nc.tensor.matmul(
                zq[:], wihT[d][:, 1, 128 * c:128 * (c + 1)],
                xq[:, qi, 1, :],
                start=False, stop=True)
            dst = zin[d][:, TPB * b_ + 2 * q:TPB * b_ + 2 * q + 2, c, :]
            if _flip[0] % 3 != 0:
                nc.scalar.activation(dst, zq[:], AF.Identity,
                                     bias=bsel[:, c:c + 1])
            else:
                nc.vector.tensor_scalar(dst, zq[:], bsel[:, c:c + 1],
                                        None, ALU.add)
            _flip[0] += 1

        # Up-front fast path: quarter-granular gathers + GEMMs for the
        # first two blocks of each direction, interleaved by need time.
        # The rest is emitted inside the scan loop (one unit per step) so
        # the zin GEMMs fill engine-idle slots instead of fighting the
        # chain at scan start.
        for qi, (d, b_, q) in enumerate(quarters):
            gather_q(qi, b_, q)
            for c in range(4):
                zin_unit_q(qi, d, b_, c, q)
        for b_ in border:
            nc.gpsimd.dma_gather(
                xT[:, b_, :, :],
                emb_d[:, :],
                idx_sb[:, 32 * b_:32 * (b_ + 1)],
                512, 512, EMB,
                transpose=True,
            )

        # Deferred-block zin units: upfront emission measured best — the
        # copyback intrusions concentrate in the first ~20 scan steps and
        # the rest of the scan runs at the clean 2213ns chain period.
        # (Spreading them through the loop locked a +437ns/step regime.)
        for j in range(2, nb - 2):
            for d, b_ in (("f", j), ("b", nb - 1 - j)):
                for c in range(4):
                    zin_unit(d, b_, c)

        def inloop_units(k):
            return []

        # ---- the recurrent scan (fwd + bwd interleaved) ----
        with tc.tile_pool(name="spsum", bufs=4, space="PSUM") as spsum, \
             tc.tile_pool(name="sQ", bufs=10) as sQ, \
             tc.tile_pool(name="sP", bufs=10) as sP, \
             tc.tile_pool(name="sT", bufs=8) as sT:

            def new_z(k):
                """Fresh psum tiles for step k with zin injected (identity mm).
                Emitted one step ahead so gate mms fire as soon as h lands."""
                zt = {}
                for d in "fb":
                    ti = k if d == "f" else nts - 1 - k
                    zt[d] = spsum.tile([128, 4, B], F32, tag="z", name=f"z{d}")
                    nc.tensor.matmul(zt[d][:], ident[:],
                                     zin[d][:, ti, :, :],
                                     start=True, stop=False)
                return zt

            q = {d: sQ.tile([128, 5, B], F32, tag="q", name=f"q{d}") for d in "fb"}
            for d in "fb":
                nc.gpsimd.memset(q[d][:, 4, :], 0.0)
            z = new_z(0)
            for k in range(steps):
                for d, rd_col in (("f", k), ("b", steps + 1 - k)):
                    for c in range(4):
                        nc.tensor.matmul(
                            z[d][:, c, :],
                            whhT[d][:, 128 * c:128 * (c + 1)],
                            hcol(d, rd_col),
                            start=False, stop=(c == 3))
                z_cur, z = z, (new_z(k + 1) if k + 1 < steps else None)
                qn = {d: sQ.tile([128, 5, B], F32, tag="q", name=f"q{d}")
                      for d in "fb"}
                # Stage-interleaved emission: both dirs' ops alternate at
                # each chain stage so the engine queues lock the two chains
                # half a period out of phase.
                wr_col = {"f": k + 1, "b": steps - k}
                P = {}
                for d in "fb":
                    # T = tanh(z') into blocks [o,i,f,g]; state cc = 2*c
                    # sits in block 4.
                    nc.scalar.activation(q[d][:, 0:4, :], z_cur[d][:],
                                         AF.Tanh)
                for d in "fb":
                    # P = [(T_i+1)*T_g | (T_f+1)*cc]
                    P[d] = sP.tile([128, 2, B], F32, tag="P", name="P")
                    nc.vector.scalar_tensor_tensor(
                        P[d][:], q[d][:, 1:3, :], 1.0, q[d][:, 3:5, :],
                        ALU.add, ALU.mult)
                for d in "fb":
                    # cc_new = 2*c_new = 0.5*P1 + P0, written straight into
                    # the next step's Q tile (no separate state-fix op).
                    nc.vector.scalar_tensor_tensor(
                        qn[d][:, 4, :], P[d][:, 1, :], 0.5, P[d][:, 0, :],
                        ALU.mult, ALU.add)
                TC = {}
                for d in "fb":
                    TC[d] = sT.tile([128, B], F32, tag="TC", name="TC")
                    nc.scalar.activation(TC[d][:], qn[d][:, 4, :], AF.Tanh,
                                         scale=0.5)
                for d in "fb":
                    # h' = (T_o + 1) * TC
                    nc.vector.scalar_tensor_tensor(
                        hcol(d, wr_col[d]), q[d][:, 0, :], 1.0, TC[d][:],
                        ALU.add, ALU.mult)
                q = qn
                for (ud, ub, uc) in inloop_units(k):
                    zin_unit(ud, ub, uc)

        # ---- emission + CRF broadcast-add + store ----
        # chunk n covers local times 2n, 2n+1 (128 tokens);
        # hf cols W+1+2n..W+2+2n, hb cols 2n+1..2n+2.
        nchunks = ch // 2
        order = sorted(range(nchunks),
                       key=lambda n: max(W + 2 + 2 * n, steps - 1 - 2 * n))
        if True:
            for n in order:
                e = epsum.tile([128, TAGS], F32, tag="e")
                nc.tensor.matmul(e[:], hspan2("f", W + 1 + 2 * n),
                                 woutT[:, 0, :], start=True, stop=False)
                nc.tensor.matmul(e[:], hspan2("b", 1 + 2 * n),
                                 woutT[:, 1, :], start=False, stop=True)
                crf_sb = ecrf.tile([128, TAGS * TAGS], F32, tag="crf")
                e_b = e[:, None, :].to_broadcast([128, TAGS, TAGS])
                nc.vector.tensor_tensor(crf_sb[:], e_b, trans[:], ALU.add)
                nc.sync.dma_start(crf_d[128 * n:128 * (n + 1), :], crf_sb[:])

    nc.compile()
    _assert_ldw_pairing(nc)
    return nc


def _assert_ldw_pairing(nc):
    """Every non-self-loading matmul must directly follow an InstLdweights
    whose weights AP matches the matmul's weights operand."""
    for f in nc.m.functions:
        for bb in f.blocks:
            prev_pe = None
            for ins in bb.instructions:
                if ins.engine != mybir.EngineType.PE:
                    continue
                if isinstance(ins, mybir.InstMatmult) and ins.ldweights is False:
                    assert isinstance(prev_pe, mybir.InstLdweights), (
                        f"{ins.name}: non-self-loading matmul not preceded by "
                        f"ldweights (got {type(prev_pe).__name__})")
                    assert repr(prev_pe.ins[0]) == repr(ins.ins[1]), (
                        f"{ins.name}: weights mismatch with {prev_pe.name}")
                prev_pe = ins


_CACHE = {}


def _get_nc():
    if "nc" not in _CACHE:
        _CACHE["nc"] = build()
    return _CACHE["nc"]


def _prep_dir(w_ih, w_hh, b):
    """Permute gates to [o,i,f,g]; apply tanh-half trick (f,i,o rows x0.5)
    and h'=2h compensation (all Whh x0.5)."""
    w_ih = np.asarray(w_ih, np.float32)[_PERM] * _BLK_SCALE[:, None]
    w_hh = np.asarray(w_hh, np.float32)[_PERM] * (0.5 * _BLK_SCALE[:, None])
    b = np.asarray(b, np.float32)[_PERM] * _BLK_SCALE
    wihT = np.ascontiguousarray(w_ih.T).astype(ml_dtypes.bfloat16)
    whhT = np.ascontiguousarray(w_hh.T).astype(ml_dtypes.bfloat16)
    bias = np.ascontiguousarray(b.reshape(4, 128).T).astype(np.float32)
    return wihT, whhT, bias


def make_in_maps(sentences, embedding, W_ih_f, W_hh_f, b_f, W_ih_b, W_hh_b,
                 b_b, W_out, b_out, transition):
    emb = np.zeros((VPAD, EMB), np.float32)
    emb[:VOCAB] = np.asarray(embedding, np.float32)
    emb = emb.astype(ml_dtypes.bfloat16)
    wihT_f, whhT_f, bias_f = _prep_dir(W_ih_f, W_hh_f, b_f)
    wihT_b, whhT_b, bias_b = _prep_dir(W_ih_b, W_hh_b, b_b)
    wo = np.asarray(W_out, np.float32) * 0.5   # h' = 2h compensation
    woutT = np.stack([np.ascontiguousarray(wo[:, :128].T),
                      np.ascontiguousarray(wo[:, 128:].T)])
    woutT = woutT.astype(ml_dtypes.bfloat16)  # [2, 128, 16]
    trans_aug = (np.asarray(transition, np.float32)
                 + np.asarray(b_out, np.float32)[None, :]).reshape(-1)  # [256]
    trans_rep = np.ascontiguousarray(
        np.broadcast_to(trans_aug, (128, 256))).astype(np.float32)
    ident = np.eye(128, dtype=ml_dtypes.bfloat16)
    zeros4 = np.zeros((128, 4), np.float32)

    # tokens per core: times [64c - W, 64c + 64 + W), batch-inner (t, b)
    # order; out-of-range times -> the zero embedding row (ZTOK).
    sent = np.asarray(sentences).astype(np.int64)  # [B, T]
    in_maps = []
    for c in range(NCORES):
        t_lo = CH * c - W
        times = np.arange(t_lo, t_lo + NTS)
        cols = np.clip(times, 0, T - 1)
        toks = sent[:, cols].T.copy()          # [NTS, B]
        toks[(times < 0) | (times >= T)] = ZTOK
        toks = toks.reshape(-1)                # (t, b) order, [NTOK]
        idx = np.tile(toks.reshape(NTOK // 16, 16).T.astype(np.int16), (8, 1))
        in_maps.append({
            "emb": emb, "idx": idx,
            "wihT_f": wihT_f, "wihT_b": wihT_b,
            "whhT_f": whhT_f, "whhT_b": whhT_b,
            "bias_f": bias_f, "bias_b": bias_b,
            "biasw_f": zeros4 if c == 0 else bias_f,
            "biasw_b": zeros4 if c == NCORES - 1 else bias_b,
            "woutT": woutT, "trans": trans_rep, "ident": ident,
        })
    return in_maps


def assemble_out(results):
    out = np.empty((B, T, TAGS, TAGS), np.float32)
    for c in range(NCORES):
        crf = results[c]["crf"].reshape(CH, B, TAGS, TAGS)
        out[:, CH * c:CH * (c + 1)] = crf.transpose(1, 0, 2, 3)
    return out


def kernel(**inputs):
    _ensure_ntff_hook()
    nc = _get_nc()
    in_maps = make_in_maps(**inputs)
    res = run_bass_kernel_spmd(nc, in_maps, list(range(NCORES)))
    return assemble_out(res.results)


# revision 41
# speedup vs baseline: 1.0476x; 1.0444x over previous
"""BiLSTM-CRF Trainium2 kernel — 8-core TIME-chunked scan.

Contract: kernel(**inputs) takes the FULL unsharded inputs (numpy arrays,
keyed as in reference.setup_inputs()) and returns the FULL [B, T, TAGS, TAGS]
crf_scores array.

Sharding: the 512-step scan is latency-bound (per-step serial chain of
~2us across PE->ACT->VEC->ACT->VEC), and per-instruction costs are almost
entirely fixed overhead, so batching all 64 sequences into one op costs
barely more than 8.  We therefore split TIME, not batch: core c computes
time chunk [64c, 64c+64) for the full batch, running its fwd/bwd scans W
extra warmup steps from zero state.  The LSTM here is strongly contractive
(forget gate ~0.5), so the warmup truncation error decays ~0.5^W.

Cell math is reformulated tanh-only (sigma(x) = (tanh(x/2)+1)/2, with
weights pre-scaled on host, hidden state carried as h' = 2h and cell
state as cc = 2c):
    T = tanh(z')            z' blocks [o, i, f, g] with f,i,o halved
    P = [(T_i+1)*T_g | (T_f+1)*cc]      (one scalar_tensor_tensor)
    cc_new = 0.5*P_f + P_i              (one scalar_tensor_tensor,
                                         written into next step's Q tile)
    TC = tanh(0.5*cc_new)               (ACT free scale)
    h' = (T_o+1)*TC                     (one scalar_tensor_tensor)
This drops one VEC op + one ACT op vs the plain sigmoid chain and keeps
the per-step serial path at PE -> ACT -> VEC -> VEC -> ACT -> VEC.
"""
import sys
import types
from contextlib import ExitStack

import ml_dtypes
import numpy as np

import concourse.bacc as bacc
import concourse.bass as bass
import concourse.mybir as mybir
import concourse.tile as tile
from concourse import library_config
from concourse.bass_utils import run_bass_kernel_spmd

# ---- problem dims (hardcoded per spec) ----
VOCAB = 30000
VPAD = 30720      # emb table padded; row 30000 is all-zeros (warmup token)
ZTOK = 30000
EMB = 256
HD = 128          # per-direction hidden
G4 = 512          # 4*HD gates
TAGS = 16
B, T = 64, 512
NCORES = 8

CH = 64           # time chunk per core
W = 16            # warmup steps
S = CH + W        # scan steps per direction
NTS = CH + 2 * W  # timesteps of tokens/zin per core (union fwd+bwd windows)
NTOK = NTS * B    # tokens per core
TPB = 8           # timesteps per 512-token block
NB = NTS // TPB   # gather/gemm blocks
WB = W // TPB     # warmup blocks per end

BF16 = mybir.dt.bfloat16
F32 = mybir.dt.float32
I16 = mybir.dt.int16
AF = mybir.ActivationFunctionType
ALU = mybir.AluOpType

# gate order in reference (jnp.split): i, f, g, o.  Device block order is
# [o, i, f, g] so that W-pair = [T_i|T_f] and V-pair = [T_g|c] are both
# contiguous ascending (c lives in block 4 of the Q tile).
_PERM = np.concatenate([
    np.arange(384, 512),   # o
    np.arange(0, 128),     # i
    np.arange(128, 256),   # f
    np.arange(256, 384),   # g
])
# per-block extra scale for the tanh-half trick: f,i,o rows halved; g not.
_BLK_SCALE = np.concatenate([
    np.full(128, 0.5),     # o
    np.full(128, 0.5),     # i
    np.full(128, 0.5),     # f
    np.full(128, 1.0),     # g
])


def _ensure_ntff_hook():
    """The RL image's antenv lacks axon_hooks; inject it so trace=True works."""
    if "antenv.axon_hooks" in sys.modules:
        return
    mod = types.ModuleType("antenv.axon_hooks")
    mod._hook = None
    mod.set_axon_ntff_profile_hook = lambda h: setattr(mod, "_hook", h)
    mod.get_axon_ntff_profile_hook = lambda: mod._hook
    sys.modules["antenv.axon_hooks"] = mod
    try:
        import antenv
        antenv.axon_hooks = mod
        from trn_agent_boot.trn_boot import _ntff_profile_via_ctypes
        mod.set_axon_ntff_profile_hook(
            _ntff_profile_via_ctypes("/opt/axon/libaxon_pjrt.so"))
    except Exception:
        pass


def build(steps: int = S, nts: int = NTS):
    """Build + compile the per-core Bass program."""
    nb = nts // TPB
    ntok = nts * B
    ch = steps - W
    nc = bacc.Bacc("TRN2", target_bir_lowering=False, debug=False)

    # ---- DRAM I/O ----
    emb_d = nc.dram_tensor("emb", [VPAD, EMB], BF16, kind="ExternalInput")
    idx_d = nc.dram_tensor("idx", [128, ntok // 16], I16, kind="ExternalInput")
    wihT_d = {d: nc.dram_tensor(f"wihT_{d}", [EMB, G4], BF16, kind="ExternalInput")
              for d in "fb"}
    whhT_d = {d: nc.dram_tensor(f"whhT_{d}", [HD, G4], BF16, kind="ExternalInput")
              for d in "fb"}
    # main bias + warmup-window bias (zeroed on edge cores), [128, 4] f32
    bias_d = {d: nc.dram_tensor(f"bias_{d}", [128, 4], F32, kind="ExternalInput")
              for d in "fb"}
    biasw_d = {d: nc.dram_tensor(f"biasw_{d}", [128, 4], F32, kind="ExternalInput")
               for d in "fb"}
    woutT_d = nc.dram_tensor("woutT", [2, HD, TAGS], BF16, kind="ExternalInput")
    trans_d = nc.dram_tensor("trans", [128, TAGS * TAGS], F32, kind="ExternalInput")
    ident_d = nc.dram_tensor("ident", [128, 128], BF16, kind="ExternalInput")
    crf_d = nc.dram_tensor("crf", [ch * B, TAGS * TAGS], F32, kind="ExternalOutput")

    with tile.TileContext(nc) as tc, ExitStack() as ctx:
        nc.gpsimd.load_library(library_config.mlp)
        const = ctx.enter_context(tc.tile_pool(name="const", bufs=1))
        big = ctx.enter_context(tc.tile_pool(name="big", bufs=1))
        # emission pools opened early so their PSUM banks / SBUF slots are
        # disjoint from the scan pools (no pool-release serialization).
        epsum = ctx.enter_context(tc.tile_pool(name="epsum", bufs=2, space="PSUM"))
        ecrf = ctx.enter_context(tc.tile_pool(name="ecrf", bufs=4))

        # ---- persistent SBUF ----
        idx_sb = const.tile([128, ntok // 16], I16)
        wihT = {d: const.tile([128, 2, G4], BF16, tag=f"wihT{d}", name=f"wihT{d}") for d in "fb"}
        whhT = {d: const.tile([HD, G4], BF16, tag=f"whhT{d}", name=f"whhT{d}") for d in "fb"}
        bias = {d: const.tile([128, 4], F32, tag=f"bias{d}", name=f"bias{d}") for d in "fb"}
        biasw = {d: const.tile([128, 4], F32, tag=f"biasw{d}", name=f"biasw{d}") for d in "fb"}
        woutT = const.tile([HD, 2, TAGS], BF16)
        trans = const.tile([128, TAGS * TAGS], F32)
        ident = const.tile([128, 128], BF16)

        # token embeddings, transposed: [128 emb-part, block, emb-half, 512 tok]
        xT = big.tile([128, nb, 2, 512], BF16, tag="xT")
        # quarter-granular staging for the fast-path first blocks (gather
        # out APs must be contiguous, so quarters get their own slots)
        xq = big.tile([128, 16, 2, 128], BF16, tag="xq")
        # input projections, [128 gate-part, t, block(oifg), batch]
        zin = {d: big.tile([128, nts, 4, B], BF16, tag=f"zin{d}", name=f"zin{d}")
               for d in "fb"}
        # h' histories (bf16), split into 16-col segments so the emission
        # epilogue's tile-granular deps bind to a segment (overlaps the
        # scan) instead of the whole history.
        # fwd: col k+1 = h' after fwd step k; real time t0+i at col W+1+i.
        # bwd: step j writes col steps-j (descending); real time t0+i at
        #   col i+1; col steps+1 is the zero init.
        # Segments: [0..W], then 16-col segments covering the real cols.
        hbnd = [0, W + 1] + [W + 1 + 16 * i for i in range(1, ch // 16 + 1)] \
            + [steps + 2]
        hseg = {d: [big.tile([128, hbnd[i + 1] - hbnd[i], B], BF16,
                             tag=f"h{d}{i}", name=f"h{d}{i}")
                    for i in range(len(hbnd) - 1)] for d in "fb"}

        def hcol(d, col):
            for i in range(len(hbnd) - 1):
                if col < hbnd[i + 1]:
                    return hseg[d][i][:, col - hbnd[i], :]
            raise AssertionError(col)

        def hspan2(d, c0):
            """[128, 2, B] span over cols c0, c0+1 (same segment)."""
            for i in range(len(hbnd) - 1):
                if c0 < hbnd[i + 1]:
                    assert c0 + 2 <= hbnd[i + 1], (d, c0)
                    return hseg[d][i][:, c0 - hbnd[i]:c0 - hbnd[i] + 2, :]
            raise AssertionError(c0)

        # ---- load constants / inputs ----
        nc.sync.dma_start(idx_sb[:], idx_d[:])
        for d in "fb":
            nc.sync.dma_start(wihT[d][:], wihT_d[d].rearrange("(k p) g -> p k g", p=128))
            nc.sync.dma_start(whhT[d][:], whhT_d[d][:])
            nc.sync.dma_start(bias[d][:], bias_d[d][:])
            nc.sync.dma_start(biasw[d][:], biasw_d[d][:])
        nc.sync.dma_start(woutT[:], woutT_d.rearrange("c h t -> h c t"))
        nc.sync.dma_start(trans[:], trans_d[:])
        nc.sync.dma_start(ident[:], ident_d[:])
        nc.gpsimd.memset(hcol("f", 0), 0.0)
        nc.gpsimd.memset(hcol("b", steps + 1), 0.0)

        # ---- embedding gather straight into x.T layout ----
        # fwd consumes blocks ascending from 0, bwd descending from nb-1.
        # The first two blocks of each direction are gathered in 128-token
        # quarters, interleaved by first-use time, so the scan can start
        # ~17us earlier (it only needs f:(0,q0) and b:(nb-1,q3) up front).
        def gather_q(qi, b_, q):
            nc.gpsimd.dma_gather(
                xq[:, qi, :, :],
                emb_d[:, :],
                idx_sb[:, 32 * b_ + 8 * q:32 * b_ + 8 * (q + 1)],
                128, 128, EMB,
                transpose=True,
            )

        quarters = []
        for j in range(8):
            quarters.append(("f", j // 4, j % 4))          # blocks 0,1 fwd
            quarters.append(("b", nb - 1 - j // 4, 3 - j % 4))  # nb-1, nb-2
        border = []
        for k in range(2, (nb + 1) // 2):
            border.append(k)
            if nb - 1 - k != k:
                border.append(nb - 1 - k)

        # ---- input projections: zin = x @ Wih.T + b ----
        # fwd needs blocks [0, nb-1-WB), bwd needs [WB, nb).  Warmup-window
        # blocks (first WB for fwd, last WB for bwd) use the biasw tensor.
        zpsum = ctx.enter_context(tc.tile_pool(name="zpsum", bufs=2, space="PSUM"))
        _flip = [0]

        def zin_unit(d, b_, c):
            """One (dir, block, gate-slice) GEMM + bias copyback."""
            warm = (d == "f" and b_ < WB) or (d == "b" and b_ >= nb - WB)
            bsel = biasw[d] if warm else bias[d]
            zp = zpsum.tile([128, 512], F32, tag="zp")
            nc.tensor.matmul(
                zp[:], wihT[d][:, 0, 128 * c:128 * (c + 1)],
                xT[:, b_, 0, :],
                start=True, stop=False)
            nc.tensor.matmul(
                zp[:], wihT[d][:, 1, 128 * c:128 * (c + 1)],
                xT[:, b_, 1, :],
                start=False, stop=True)
            # strided copyback into [t, c, b] layout, bias folded in;
            # alternate ACT/VEC to balance engine load.
            dst = zin[d][:, TPB * b_:TPB * (b_ + 1), c, :]
            if _flip[0] % 2 == 0:
                nc.scalar.activation(dst, zp[:], AF.Identity,
                                     bias=bsel[:, c:c + 1])
            else:
                nc.vector.tensor_scalar(dst, zp[:], bsel[:, c:c + 1],
                                        None, ALU.add)
            _flip[0] += 1

        def zin_unit_q(qi, d, b_, c, q):
            """Quarter-block (128-token) GEMM + bias copyback."""
            warm = (d == "f" and b_ < WB) or (d == "b" and b_ >= nb - WB)
            bsel = biasw[d] if warm else bias[d]
            zq = zpsum.tile([128, 128], F32, tag="zp")
            nc.tensor.matmul(
                zq[:], wihT[d][:, 0, 128 * c:128 * (c + 1)],
                xq[:, qi, 0, :],
                start=True, stop=False)
            nc.tensor.matmul(
                zq[:], wihT[d][:, 1, 128 * c:128 * (c + 1)],
                xq[:, qi, 1, :],
                start=False, stop=True)
            dst = zin[d][:, TPB * b_ + 2 * q:TPB * b_ + 2 * q + 2, c, :]
            if _flip[0] % 2 == 0:
                nc.scalar.activation(dst, zq[:], AF.Identity,
                                     bias=bsel[:, c:c + 1])
            else:
                nc.vector.tensor_scalar(dst, zq[:], bsel[:, c:c + 1],
                                        None, ALU.add)
            _flip[0] += 1

        # Up-front fast path: quarter-granular gathers + GEMMs for the
        # first two blocks of each direction, interleaved by need time.
        # The rest is emitted inside the scan loop (one unit per step) so
        # the zin GEMMs fill engine-idle slots instead of fighting the
        # chain at scan start.
        for qi, (d, b_, q) in enumerate(quarters):
            gather_q(qi, b_, q)
            for c in range(4):
                zin_unit_q(qi, d, b_, c, q)
        for b_ in border:
            nc.gpsimd.dma_gather(
                xT[:, b_, :, :],
                emb_d[:, :],
                idx_sb[:, 32 * b_:32 * (b_ + 1)],
                512, 512, EMB,
                transpose=True,
            )

        # Deferred-block zin units: upfront emission measured best — the
        # copyback intrusions concentrate in the first ~20 scan steps and
        # the rest of the scan runs at the clean 2213ns chain period.
        # (Spreading them through the loop locked a +437ns/step regime.)
        for j in range(2, nb - 2):
            for d, b_ in (("f", j), ("b", nb - 1 - j)):
                for c in range(4):
                    zin_unit(d, b_, c)

        def inloop_units(k):
            return []

        # ---- the recurrent scan (fwd + bwd interleaved) ----
        with tc.tile_pool(name="spsum", bufs=4, space="PSUM") as spsum, \
             tc.tile_pool(name="sQ", bufs=10) as sQ, \
             tc.tile_pool(name="sP", bufs=10) as sP, \
             tc.tile_pool(name="sT", bufs=8) as sT:

            def new_z(k):
                """Fresh psum tiles for step k with zin injected (identity mm).
                Emitted one step ahead so gate mms fire as soon as h lands."""
                zt = {}
                for d in "fb":
                    ti = k if d == "f" else nts - 1 - k
                    zt[d] = spsum.tile([128, 4, B], F32, tag="z", name=f"z{d}")
                    nc.tensor.matmul(zt[d][:], ident[:],
                                     zin[d][:, ti, :, :],
                                     start=True, stop=False)
                return zt

            q = {d: sQ.tile([128, 5, B], F32, tag="q", name=f"q{d}") for d in "fb"}
            for d in "fb":
                nc.gpsimd.memset(q[d][:, 4, :], 0.0)
            z = new_z(0)
            for k in range(steps):
                for d, rd_col in (("f", k), ("b", steps + 1 - k)):
                    for c in range(4):
                        nc.tensor.matmul(
                            z[d][:, c, :],
                            whhT[d][:, 128 * c:128 * (c + 1)],
                            hcol(d, rd_col),
                            start=False, stop=(c == 3))
                z_cur, z = z, (new_z(k + 1) if k + 1 < steps else None)
                qn = {d: sQ.tile([128, 5, B], F32, tag="q", name=f"q{d}")
                      for d in "fb"}
                # Stage-interleaved emission: both dirs' ops alternate at
                # each chain stage so the engine queues lock the two chains
                # half a period out of phase.
                wr_col = {"f": k + 1, "b": steps - k}
                P = {}
                for d in "fb":
                    # T = tanh(z') into blocks [o,i,f,g]; state cc = 2*c
                    # sits in block 4.
                    nc.scalar.activation(q[d][:, 0:4, :], z_cur[d][:],
                                         AF.Tanh)
                for d in "fb":
                    # P = [(T_i+1)*T_g | (T_f+1)*cc]
                    P[d] = sP.tile([128, 2, B], F32, tag="P", name="P")
                    nc.vector.scalar_tensor_tensor(
                        P[d][:], q[d][:, 1:3, :], 1.0, q[d][:, 3:5, :],
                        ALU.add, ALU.mult)
                for d in "fb":
                    # cc_new = 2*c_new = 0.5*P1 + P0, written straight into
                    # the next step's Q tile (no separate state-fix op).
                    nc.vector.scalar_tensor_tensor(
                        qn[d][:, 4, :], P[d][:, 1, :], 0.5, P[d][:, 0, :],
                        ALU.mult, ALU.add)
                TC = {}
                for d in "fb":
                    TC[d] = sT.tile([128, B], F32, tag="TC", name="TC")
                    nc.scalar.activation(TC[d][:], qn[d][:, 4, :], AF.Tanh,
                                         scale=0.5)
                for d in "fb":
                    # h' = (T_o + 1) * TC
                    nc.vector.scalar_tensor_tensor(
                        hcol(d, wr_col[d]), q[d][:, 0, :], 1.0, TC[d][:],
                        ALU.add, ALU.mult)
                q = qn
                for (ud, ub, uc) in inloop_units(k):
                    zin_unit(ud, ub, uc)

        # ---- emission + CRF broadcast-add + store ----
        # chunk n covers local times 2n, 2n+1 (128 tokens);
        # hf cols W+1+2n..W+2+2n, hb cols 2n+1..2n+2.
        nchunks = ch // 2
        order = sorted(range(nchunks),
                       key=lambda n: max(W + 2 + 2 * n, steps - 1 - 2 * n))
        if True:
            for n in order:
                e = epsum.tile([128, TAGS], F32, tag="e")
                nc.tensor.matmul(e[:], hspan2("f", W + 1 + 2 * n),
                                 woutT[:, 0, :], start=True, stop=False)
                nc.tensor.matmul(e[:], hspan2("b", 1 + 2 * n),
                                 woutT[:, 1, :], start=False, stop=True)
                crf_sb = ecrf.tile([128, TAGS * TAGS], F32, tag="crf")
                e_b = e[:, None, :].to_broadcast([128, TAGS, TAGS])
                nc.vector.tensor_tensor(crf_sb[:], e_b, trans[:], ALU.add)
                nc.sync.dma_start(crf_d[128 * n:128 * (n + 1), :], crf_sb[:])

    nc.compile()
    _assert_ldw_pairing(nc)
    return nc


def _assert_ldw_pairing(nc):
    """Every non-self-loading matmul must directly follow an InstLdweights
    whose weights AP matches the matmul's weights operand."""
    for f in nc.m.functions:
        for bb in f.blocks:
            prev_pe = None
            for ins in bb.instructions:
                if ins.engine != mybir.EngineType.PE:
                    continue
                if isinstance(ins, mybir.InstMatmult) and ins.ldweights is False:
                    assert isinstance(prev_pe, mybir.InstLdweights), (
                        f"{ins.name}: non-self-loading matmul not preceded by "
                        f"ldweights (got {type(prev_pe).__name__})")
                    assert repr(prev_pe.ins[0]) == repr(ins.ins[1]), (
                        f"{ins.name}: weights mismatch with {prev_pe.name}")
                prev_pe = ins


_CACHE = {}


def _get_nc():
    if "nc" not in _CACHE:
        _CACHE["nc"] = build()
    return _CACHE["nc"]


def _prep_dir(w_ih, w_hh, b):
    """Permute gates to [o,i,f,g]; apply tanh-half trick (f,i,o rows x0.5)
    and h'=2h compensation (all Whh x0.5)."""
    w_ih = np.asarray(w_ih, np.float32)[_PERM] * _BLK_SCALE[:, None]
    w_hh = np.asarray(w_hh, np.float32)[_PERM] * (0.5 * _BLK_SCALE[:, None])
    b = np.asarray(b, np.float32)[_PERM] * _BLK_SCALE
    wihT = np.ascontiguousarray(w_ih.T).astype(ml_dtypes.bfloat16)
    whhT = np.ascontiguousarray(w_hh.T).astype(ml_dtypes.bfloat16)
    bias = np.ascontiguousarray(b.reshape(4, 128).T).astype(np.float32)
    return wihT, whhT, bias


def make_in_maps(sentences, embedding, W_ih_f, W_hh_f, b_f, W_ih_b, W_hh_b,
                 b_b, W_out, b_out, transition):
    emb = np.zeros((VPAD, EMB), np.float32)
    emb[:VOCAB] = np.asarray(embedding, np.float32)
    emb = emb.astype(ml_dtypes.bfloat16)
    wihT_f, whhT_f, bias_f = _prep_dir(W_ih_f, W_hh_f, b_f)
    wihT_b, whhT_b, bias_b = _prep_dir(W_ih_b, W_hh_b, b_b)
    wo = np.asarray(W_out, np.float32) * 0.5   # h' = 2h compensation
    woutT = np.stack([np.ascontiguousarray(wo[:, :128].T),
                      np.ascontiguousarray(wo[:, 128:].T)])
    woutT = woutT.astype(ml_dtypes.bfloat16)  # [2, 128, 16]
    trans_aug = (np.asarray(transition, np.float32)
                 + np.asarray(b_out, np.float32)[None, :]).reshape(-1)  # [256]
    trans_rep = np.ascontiguousarray(
        np.broadcast_to(trans_aug, (128, 256))).astype(np.float32)
    ident = np.eye(128, dtype=ml_dtypes.bfloat16)
    zeros4 = np.zeros((128, 4), np.float32)

    # tokens per core: times [64c - W, 64c + 64 + W), batch-inner (t, b)
    # order; out-of-range times -> the zero embedding row (ZTOK).
    sent = np.asarray(sentences).astype(np.int64)  # [B, T]
    in_maps = []
    for c in range(NCORES):
        t_lo = CH * c - W
        times = np.arange(t_lo, t_lo + NTS)
        cols = np.clip(times, 0, T - 1)
        toks = sent[:, cols].T.copy()          # [NTS, B]
        toks[(times < 0) | (times >= T)] = ZTOK
        toks = toks.reshape(-1)                # (t, b) order, [NTOK]
        idx = np.tile(toks.reshape(NTOK // 16, 16).T.astype(np.int16), (8, 1))
        in_maps.append({
            "emb": emb, "idx": idx,
            "wihT_f": wihT_f, "wihT_b": wihT_b,
            "whhT_f": whhT_f, "whhT_b": whhT_b,
            "bias_f": bias_f, "bias_b": bias_b,
            "biasw_f": zeros4 if c == 0 else bias_f,
            "biasw_b": zeros4 if c == NCORES - 1 else bias_b,
            "woutT": woutT, "trans": trans_rep, "ident": ident,
        })
    return in_maps


def assemble_out(results):
    out = np.empty((B, T, TAGS, TAGS), np.float32)
    for c in range(NCORES):
        crf = results[c]["crf"].reshape(CH, B, TAGS, TAGS)
        out[:, CH * c:CH * (c + 1)] = crf.transpose(1, 0, 2, 3)
    return out


def kernel(**inputs):
    _ensure_ntff_hook()
    nc = _get_nc()
    in_maps = make_in_maps(**inputs)
    res = run_bass_kernel_spmd(nc, in_maps, list(range(NCORES)))
    return assemble_out(res.results)


# revision 43
# speedup vs baseline: 1.0562x; 1.0082x over previous
"""BiLSTM-CRF Trainium2 kernel — 8-core TIME-chunked scan.

Contract: kernel(**inputs) takes the FULL unsharded inputs (numpy arrays,
keyed as in reference.setup_inputs()) and returns the FULL [B, T, TAGS, TAGS]
crf_scores array.

Sharding: the 512-step scan is latency-bound (per-step serial chain of
~2us across PE->ACT->VEC->ACT->VEC), and per-instruction costs are almost
entirely fixed overhead, so batching all 64 sequences into one op costs
barely more than 8.  We therefore split TIME, not batch: core c computes
time chunk [64c, 64c+64) for the full batch, running its fwd/bwd scans W
extra warmup steps from zero state.  The LSTM here is strongly contractive
(forget gate ~0.5), so the warmup truncation error decays ~0.5^W.

Cell math is reformulated tanh-only (sigma(x) = (tanh(x/2)+1)/2, with
weights pre-scaled on host, hidden state carried as h' = 2h and cell
state as cc = 2c):
    T = tanh(z')            z' blocks [o, i, f, g] with f,i,o halved
    P = [(T_i+1)*T_g | (T_f+1)*cc]      (one scalar_tensor_tensor)
    cc_new = 0.5*P_f + P_i              (one scalar_tensor_tensor,
                                         written into next step's Q tile)
    TC = tanh(0.5*cc_new)               (ACT free scale)
    h' = (T_o+1)*TC                     (one scalar_tensor_tensor)
This drops one VEC op + one ACT op vs the plain sigmoid chain and keeps
the per-step serial path at PE -> ACT -> VEC -> VEC -> ACT -> VEC.
"""
import sys
import types
from contextlib import ExitStack

import ml_dtypes
import numpy as np

import concourse.bacc as bacc
import concourse.bass as bass
import concourse.mybir as mybir
import concourse.tile as tile
from concourse import library_config
from concourse.bass_utils import run_bass_kernel_spmd

# ---- problem dims (hardcoded per spec) ----
VOCAB = 30000
VPAD = 30720      # emb table padded; row 30000 is all-zeros (warmup token)
ZTOK = 30000
EMB = 256
HD = 128          # per-direction hidden
G4 = 512          # 4*HD gates
TAGS = 16
B, T = 64, 512
NCORES = 8

CH = 64           # time chunk per core
W = 16            # warmup steps
S = CH + W        # scan steps per direction
NTS = CH + 2 * W  # timesteps of tokens/zin per core (union fwd+bwd windows)
NTOK = NTS * B    # tokens per core
TPB = 8           # timesteps per 512-token block
NB = NTS // TPB   # gather/gemm blocks
WB = W // TPB     # warmup blocks per end

BF16 = mybir.dt.bfloat16
F32 = mybir.dt.float32
I16 = mybir.dt.int16
AF = mybir.ActivationFunctionType
ALU = mybir.AluOpType

# gate order in reference (jnp.split): i, f, g, o.  Device block order is
# [o, i, f, g] so that W-pair = [T_i|T_f] and V-pair = [T_g|c] are both
# contiguous ascending (c lives in block 4 of the Q tile).
_PERM = np.concatenate([
    np.arange(384, 512),   # o
    np.arange(0, 128),     # i
    np.arange(128, 256),   # f
    np.arange(256, 384),   # g
])
# per-block extra scale for the tanh-half trick: f,i,o rows halved; g not.
_BLK_SCALE = np.concatenate([
    np.full(128, 0.5),     # o
    np.full(128, 0.5),     # i
    np.full(128, 0.5),     # f
    np.full(128, 1.0),     # g
])


def _ensure_ntff_hook():
    """The RL image's antenv lacks axon_hooks; inject it so trace=True works."""
    if "antenv.axon_hooks" in sys.modules:
        return
    mod = types.ModuleType("antenv.axon_hooks")
    mod._hook = None
    mod.set_axon_ntff_profile_hook = lambda h: setattr(mod, "_hook", h)
    mod.get_axon_ntff_profile_hook = lambda: mod._hook
    sys.modules["antenv.axon_hooks"] = mod
    try:
        import antenv
        antenv.axon_hooks = mod
        from trn_agent_boot.trn_boot import _ntff_profile_via_ctypes
        mod.set_axon_ntff_profile_hook(
            _ntff_profile_via_ctypes("/opt/axon/libaxon_pjrt.so"))
    except Exception:
        pass


def build(steps: int = S, nts: int = NTS):
    """Build + compile the per-core Bass program."""
    nb = nts // TPB
    ntok = nts * B
    ch = steps - W
    nc = bacc.Bacc("TRN2", target_bir_lowering=False, debug=False)

    # ---- DRAM I/O ----
    emb_d = nc.dram_tensor("emb", [VPAD, EMB], BF16, kind="ExternalInput")
    idx_d = nc.dram_tensor("idx", [128, ntok // 16], I16, kind="ExternalInput")
    wihT_d = {d: nc.dram_tensor(f"wihT_{d}", [EMB, G4], BF16, kind="ExternalInput")
              for d in "fb"}
    whhT_d = {d: nc.dram_tensor(f"whhT_{d}", [HD, G4], BF16, kind="ExternalInput")
              for d in "fb"}
    # main bias + warmup-window bias (zeroed on edge cores), [128, 4] f32
    bias_d = {d: nc.dram_tensor(f"bias_{d}", [128, 4], F32, kind="ExternalInput")
              for d in "fb"}
    biasw_d = {d: nc.dram_tensor(f"biasw_{d}", [128, 4], F32, kind="ExternalInput")
               for d in "fb"}
    woutT_d = nc.dram_tensor("woutT", [2, HD, TAGS], BF16, kind="ExternalInput")
    trans_d = nc.dram_tensor("trans", [128, TAGS * TAGS], F32, kind="ExternalInput")
    ident_d = nc.dram_tensor("ident", [128, 128], BF16, kind="ExternalInput")
    crf_d = nc.dram_tensor("crf", [ch * B, TAGS * TAGS], F32, kind="ExternalOutput")

    with tile.TileContext(nc) as tc, ExitStack() as ctx:
        nc.gpsimd.load_library(library_config.mlp)
        const = ctx.enter_context(tc.tile_pool(name="const", bufs=1))
        big = ctx.enter_context(tc.tile_pool(name="big", bufs=1))
        # emission pools opened early so their PSUM banks / SBUF slots are
        # disjoint from the scan pools (no pool-release serialization).
        epsum = ctx.enter_context(tc.tile_pool(name="epsum", bufs=2, space="PSUM"))
        ecrf = ctx.enter_context(tc.tile_pool(name="ecrf", bufs=4))

        # ---- persistent SBUF ----
        idx_sb = const.tile([128, ntok // 16], I16)
        wihT = {d: const.tile([128, 2, G4], BF16, tag=f"wihT{d}", name=f"wihT{d}") for d in "fb"}
        whhT = {d: const.tile([HD, G4], BF16, tag=f"whhT{d}", name=f"whhT{d}") for d in "fb"}
        bias = {d: const.tile([128, 4], F32, tag=f"bias{d}", name=f"bias{d}") for d in "fb"}
        biasw = {d: const.tile([128, 4], F32, tag=f"biasw{d}", name=f"biasw{d}") for d in "fb"}
        woutT = const.tile([HD, 2, TAGS], BF16)
        trans = const.tile([128, TAGS * TAGS], F32)
        ident = const.tile([128, 128], BF16)

        # token embeddings, transposed: [128 emb-part, block, emb-half, 512 tok]
        xT = big.tile([128, nb, 2, 512], BF16, tag="xT")
        # quarter-granular staging for the fast-path first blocks (gather
        # out APs must be contiguous, so quarters get their own slots)
        xq = big.tile([128, 16, 2, 128], BF16, tag="xq")
        # input projections, [128 gate-part, t, block(oifg), batch]
        zin = {d: big.tile([128, nts, 4, B], BF16, tag=f"zin{d}", name=f"zin{d}")
               for d in "fb"}
        # h' histories (bf16), split into 16-col segments so the emission
        # epilogue's tile-granular deps bind to a segment (overlaps the
        # scan) instead of the whole history.
        # fwd: col k+1 = h' after fwd step k; real time t0+i at col W+1+i.
        # bwd: step j writes col steps-j (descending); real time t0+i at
        #   col i+1; col steps+1 is the zero init.
        # Segments: [0..W], then 16-col segments covering the real cols.
        hbnd = [0, W + 1] + [W + 1 + 16 * i for i in range(1, ch // 16 + 1)] \
            + [steps + 2]
        hseg = {d: [big.tile([128, hbnd[i + 1] - hbnd[i], B], BF16,
                             tag=f"h{d}{i}", name=f"h{d}{i}")
                    for i in range(len(hbnd) - 1)] for d in "fb"}

        def hcol(d, col):
            for i in range(len(hbnd) - 1):
                if col < hbnd[i + 1]:
                    return hseg[d][i][:, col - hbnd[i], :]
            raise AssertionError(col)

        def hspan2(d, c0):
            """[128, 2, B] span over cols c0, c0+1 (same segment)."""
            for i in range(len(hbnd) - 1):
                if c0 < hbnd[i + 1]:
                    assert c0 + 2 <= hbnd[i + 1], (d, c0)
                    return hseg[d][i][:, c0 - hbnd[i]:c0 - hbnd[i] + 2, :]
            raise AssertionError(c0)

        # ---- load inputs: idx first — it is the only DMA the gathers
        # need, and the SP engine issues DMAs serially at ~650ns each, so
        # the weight/const DMAs are emitted AFTER the gathers below.
        nc.sync.dma_start(idx_sb[:], idx_d[:])
        nc.gpsimd.memset(hcol("f", 0), 0.0)
        nc.gpsimd.memset(hcol("b", steps + 1), 0.0)

        def load_consts():
            for d in "fb":
                nc.sync.dma_start(wihT[d][:], wihT_d[d].rearrange("(k p) g -> p k g", p=128))
                nc.sync.dma_start(whhT[d][:], whhT_d[d][:])
                nc.sync.dma_start(bias[d][:], bias_d[d][:])
                nc.sync.dma_start(biasw[d][:], biasw_d[d][:])
            nc.sync.dma_start(woutT[:], woutT_d.rearrange("c h t -> h c t"))
            nc.sync.dma_start(trans[:], trans_d[:])
            nc.sync.dma_start(ident[:], ident_d[:])

        # ---- embedding gather straight into x.T layout ----
        # fwd consumes blocks ascending from 0, bwd descending from nb-1.
        # The first two blocks of each direction are gathered in 128-token
        # quarters, interleaved by first-use time, so the scan can start
        # ~17us earlier (it only needs f:(0,q0) and b:(nb-1,q3) up front).
        def gather_q(qi, b_, q):
            nc.gpsimd.dma_gather(
                xq[:, qi, :, :],
                emb_d[:, :],
                idx_sb[:, 32 * b_ + 8 * q:32 * b_ + 8 * (q + 1)],
                128, 128, EMB,
                transpose=True,
            )

        quarters = []
        for j in range(8):
            quarters.append(("f", j // 4, j % 4))          # blocks 0,1 fwd
            quarters.append(("b", nb - 1 - j // 4, 3 - j % 4))  # nb-1, nb-2
        border = []
        for k in range(2, (nb + 1) // 2):
            border.append(k)
            if nb - 1 - k != k:
                border.append(nb - 1 - k)

        # ---- input projections: zin = x @ Wih.T + b ----
        # fwd needs blocks [0, nb-1-WB), bwd needs [WB, nb).  Warmup-window
        # blocks (first WB for fwd, last WB for bwd) use the biasw tensor.
        zpsum = ctx.enter_context(tc.tile_pool(name="zpsum", bufs=2, space="PSUM"))
        _flip = [0]

        def zin_unit(d, b_, c):
            """One (dir, block, gate-slice) GEMM + bias copyback."""
            warm = (d == "f" and b_ < WB) or (d == "b" and b_ >= nb - WB)
            bsel = biasw[d] if warm else bias[d]
            zp = zpsum.tile([128, 512], F32, tag="zp")
            nc.tensor.matmul(
                zp[:], wihT[d][:, 0, 128 * c:128 * (c + 1)],
                xT[:, b_, 0, :],
                start=True, stop=False)
            nc.tensor.matmul(
                zp[:], wihT[d][:, 1, 128 * c:128 * (c + 1)],
                xT[:, b_, 1, :],
                start=False, stop=True)
            # strided copyback into [t, c, b] layout, bias folded in;
            # alternate ACT/VEC to balance engine load.
            dst = zin[d][:, TPB * b_:TPB * (b_ + 1), c, :]
            if _flip[0] % 2 == 0:
                nc.scalar.activation(dst, zp[:], AF.Identity,
                                     bias=bsel[:, c:c + 1])
            else:
                nc.vector.tensor_scalar(dst, zp[:], bsel[:, c:c + 1],
                                        None, ALU.add)
            _flip[0] += 1

        def zin_unit_q(qi, d, b_, c, q):
            """Quarter-block (128-token) GEMM + bias copyback."""
            warm = (d == "f" and b_ < WB) or (d == "b" and b_ >= nb - WB)
            bsel = biasw[d] if warm else bias[d]
            zq = zpsum.tile([128, 128], F32, tag="zp")
            nc.tensor.matmul(
                zq[:], wihT[d][:, 0, 128 * c:128 * (c + 1)],
                xq[:, qi, 0, :],
                start=True, stop=False)
            nc.tensor.matmul(
                zq[:], wihT[d][:, 1, 128 * c:128 * (c + 1)],
                xq[:, qi, 1, :],
                start=False, stop=True)
            dst = zin[d][:, TPB * b_ + 2 * q:TPB * b_ + 2 * q + 2, c, :]
            if _flip[0] % 2 == 0:
                nc.scalar.activation(dst, zq[:], AF.Identity,
                                     bias=bsel[:, c:c + 1])
            else:
                nc.vector.tensor_scalar(dst, zq[:], bsel[:, c:c + 1],
                                        None, ALU.add)
            _flip[0] += 1

        # Up-front fast path: quarter-granular gathers + GEMMs for the
        # first two blocks of each direction, interleaved by need time.
        # The rest is emitted inside the scan loop (one unit per step) so
        # the zin GEMMs fill engine-idle slots instead of fighting the
        # chain at scan start.
        for qi, (d, b_, q) in enumerate(quarters):
            gather_q(qi, b_, q)
        load_consts()
        for qi, (d, b_, q) in enumerate(quarters):
            for c in range(4):
                zin_unit_q(qi, d, b_, c, q)
        for b_ in border:
            nc.gpsimd.dma_gather(
                xT[:, b_, :, :],
                emb_d[:, :],
                idx_sb[:, 32 * b_:32 * (b_ + 1)],
                512, 512, EMB,
                transpose=True,
            )

        # Deferred-block zin units: upfront emission measured best — the
        # copyback intrusions concentrate in the first ~20 scan steps and
        # the rest of the scan runs at the clean 2213ns chain period.
        # (Spreading them through the loop locked a +437ns/step regime.)
        for j in range(2, nb - 2):
            for d, b_ in (("f", j), ("b", nb - 1 - j)):
                for c in range(4):
                    zin_unit(d, b_, c)

        def inloop_units(k):
            return []

        # ---- the recurrent scan (fwd + bwd interleaved) ----
        with tc.tile_pool(name="spsum", bufs=4, space="PSUM") as spsum, \
             tc.tile_pool(name="sQ", bufs=10) as sQ, \
             tc.tile_pool(name="sP", bufs=10) as sP, \
             tc.tile_pool(name="sT", bufs=8) as sT:

            def new_z(k):
                """Fresh psum tiles for step k with zin injected (identity mm).
                Emitted one step ahead so gate mms fire as soon as h lands."""
                zt = {}
                for d in "fb":
                    ti = k if d == "f" else nts - 1 - k
                    zt[d] = spsum.tile([128, 4, B], F32, tag="z", name=f"z{d}")
                    nc.tensor.matmul(zt[d][:], ident[:],
                                     zin[d][:, ti, :, :],
                                     start=True, stop=False)
                return zt

            q = {d: sQ.tile([128, 5, B], F32, tag="q", name=f"q{d}") for d in "fb"}
            for d in "fb":
                nc.gpsimd.memset(q[d][:, 4, :], 0.0)
            z = new_z(0)
            for k in range(steps):
                for d, rd_col in (("f", k), ("b", steps + 1 - k)):
                    for c in range(4):
                        nc.tensor.matmul(
                            z[d][:, c, :],
                            whhT[d][:, 128 * c:128 * (c + 1)],
                            hcol(d, rd_col),
                            start=False, stop=(c == 3))
                z_cur, z = z, (new_z(k + 1) if k + 1 < steps else None)
                qn = {d: sQ.tile([128, 5, B], F32, tag="q", name=f"q{d}")
                      for d in "fb"}
                # Stage-interleaved emission: both dirs' ops alternate at
                # each chain stage so the engine queues lock the two chains
                # half a period out of phase.
                wr_col = {"f": k + 1, "b": steps - k}
                P = {}
                for d in "fb":
                    # T = tanh(z') into blocks [o,i,f,g]; state cc = 2*c
                    # sits in block 4.
                    nc.scalar.activation(q[d][:, 0:4, :], z_cur[d][:],
                                         AF.Tanh)
                for d in "fb":
                    # P = [(T_i+1)*T_g | (T_f+1)*cc]
                    P[d] = sP.tile([128, 2, B], F32, tag="P", name="P")
                    nc.vector.scalar_tensor_tensor(
                        P[d][:], q[d][:, 1:3, :], 1.0, q[d][:, 3:5, :],
                        ALU.add, ALU.mult)
                for d in "fb":
                    # cc_new = 2*c_new = 0.5*P1 + P0, written straight into
                    # the next step's Q tile (no separate state-fix op).
                    nc.vector.scalar_tensor_tensor(
                        qn[d][:, 4, :], P[d][:, 1, :], 0.5, P[d][:, 0, :],
                        ALU.mult, ALU.add)
                TC = {}
                for d in "fb":
                    TC[d] = sT.tile([128, B], F32, tag="TC", name="TC")
                    nc.scalar.activation(TC[d][:], qn[d][:, 4, :], AF.Tanh,
                                         scale=0.5)
                for d in "fb":
                    # h' = (T_o + 1) * TC
                    nc.vector.scalar_tensor_tensor(
                        hcol(d, wr_col[d]), q[d][:, 0, :], 1.0, TC[d][:],
                        ALU.add, ALU.mult)
                q = qn
                for (ud, ub, uc) in inloop_units(k):
                    zin_unit(ud, ub, uc)

        # ---- emission + CRF broadcast-add + store ----
        # chunk n covers local times 2n, 2n+1 (128 tokens);
        # hf cols W+1+2n..W+2+2n, hb cols 2n+1..2n+2.
        nchunks = ch // 2
        order = sorted(range(nchunks),
                       key=lambda n: max(W + 2 + 2 * n, steps - 1 - 2 * n))
        if True:
            for n in order:
                e = epsum.tile([128, TAGS], F32, tag="e")
                nc.tensor.matmul(e[:], hspan2("f", W + 1 + 2 * n),
                                 woutT[:, 0, :], start=True, stop=False)
                nc.tensor.matmul(e[:], hspan2("b", 1 + 2 * n),
                                 woutT[:, 1, :], start=False, stop=True)
                crf_sb = ecrf.tile([128, TAGS * TAGS], F32, tag="crf")
                e_b = e[:, None, :].to_broadcast([128, TAGS, TAGS])
                nc.vector.tensor_tensor(crf_sb[:], e_b, trans[:], ALU.add)
                nc.sync.dma_start(crf_d[128 * n:128 * (n + 1), :], crf_sb[:])

    nc.compile()
    _assert_ldw_pairing(nc)
    return nc


def _assert_ldw_pairing(nc):
    """Every non-self-loading matmul must directly follow an InstLdweights
    whose weights AP matches the matmul's weights operand."""
    for f in nc.m.functions:
        for bb in f.blocks:
            prev_pe = None
            for ins in bb.instructions:
                if ins.engine != mybir.EngineType.PE:
                    continue
                if isinstance(ins, mybir.InstMatmult) and ins.ldweights is False:
                    assert isinstance(prev_pe, mybir.InstLdweights), (
                        f"{ins.name}: non-self-loading matmul not preceded by "
                        f"ldweights (got {type(prev_pe).__name__})")
                    assert repr(prev_pe.ins[0]) == repr(ins.ins[1]), (
                        f"{ins.name}: weights mismatch with {prev_pe.name}")
                prev_pe = ins


_CACHE = {}


def _get_nc():
    if "nc" not in _CACHE:
        _CACHE["nc"] = build()
    return _CACHE["nc"]


def _prep_dir(w_ih, w_hh, b):
    """Permute gates to [o,i,f,g]; apply tanh-half trick (f,i,o rows x0.5)
    and h'=2h compensation (all Whh x0.5)."""
    w_ih = np.asarray(w_ih, np.float32)[_PERM] * _BLK_SCALE[:, None]
    w_hh = np.asarray(w_hh, np.float32)[_PERM] * (0.5 * _BLK_SCALE[:, None])
    b = np.asarray(b, np.float32)[_PERM] * _BLK_SCALE
    wihT = np.ascontiguousarray(w_ih.T).astype(ml_dtypes.bfloat16)
    whhT = np.ascontiguousarray(w_hh.T).astype(ml_dtypes.bfloat16)
    bias = np.ascontiguousarray(b.reshape(4, 128).T).astype(np.float32)
    return wihT, whhT, bias


def make_in_maps(sentences, embedding, W_ih_f, W_hh_f, b_f, W_ih_b, W_hh_b,
                 b_b, W_out, b_out, transition):
    emb = np.zeros((VPAD, EMB), np.float32)
    emb[:VOCAB] = np.asarray(embedding, np.float32)
    emb = emb.astype(ml_dtypes.bfloat16)
    wihT_f, whhT_f, bias_f = _prep_dir(W_ih_f, W_hh_f, b_f)
    wihT_b, whhT_b, bias_b = _prep_dir(W_ih_b, W_hh_b, b_b)
    wo = np.asarray(W_out, np.float32) * 0.5   # h' = 2h compensation
    woutT = np.stack([np.ascontiguousarray(wo[:, :128].T),
                      np.ascontiguousarray(wo[:, 128:].T)])
    woutT = woutT.astype(ml_dtypes.bfloat16)  # [2, 128, 16]
    trans_aug = (np.asarray(transition, np.float32)
                 + np.asarray(b_out, np.float32)[None, :]).reshape(-1)  # [256]
    trans_rep = np.ascontiguousarray(
        np.broadcast_to(trans_aug, (128, 256))).astype(np.float32)
    ident = np.eye(128, dtype=ml_dtypes.bfloat16)
    zeros4 = np.zeros((128, 4), np.float32)

    # tokens per core: times [64c - W, 64c + 64 + W), batch-inner (t, b)
    # order; out-of-range times -> the zero embedding row (ZTOK).
    sent = np.asarray(sentences).astype(np.int64)  # [B, T]
    in_maps = []
    for c in range(NCORES):
        t_lo = CH * c - W
        times = np.arange(t_lo, t_lo + NTS)
        cols = np.clip(times, 0, T - 1)
        toks = sent[:, cols].T.copy()          # [NTS, B]
        toks[(times < 0) | (times >= T)] = ZTOK
        toks = toks.reshape(-1)                # (t, b) order, [NTOK]
        idx = np.tile(toks.reshape(NTOK // 16, 16).T.astype(np.int16), (8, 1))
        in_maps.append({
            "emb": emb, "idx": idx,
            "wihT_f": wihT_f, "wihT_b": wihT_b,
            "whhT_f": whhT_f, "whhT_b": whhT_b,
            "bias_f": bias_f, "bias_b": bias_b,
            "biasw_f": zeros4 if c == 0 else bias_f,
            "biasw_b": zeros4 if c == NCORES - 1 else bias_b,
            "woutT": woutT, "trans": trans_rep, "ident": ident,
        })
    return in_maps


def assemble_out(results):
    out = np.empty((B, T, TAGS, TAGS), np.float32)
    for c in range(NCORES):
        crf = results[c]["crf"].reshape(CH, B, TAGS, TAGS)
        out[:, CH * c:CH * (c + 1)] = crf.transpose(1, 0, 2, 3)
    return out


def kernel(**inputs):
    _ensure_ntff_hook()
    nc = _get_nc()
    in_maps = make_in_maps(**inputs)
    res = run_bass_kernel_spmd(nc, in_maps, list(range(NCORES)))
    return assemble_out(res.results)


# revision 44
# speedup vs baseline: 1.0593x; 1.0029x over previous
"""BiLSTM-CRF Trainium2 kernel — 8-core TIME-chunked scan.

Contract: kernel(**inputs) takes the FULL unsharded inputs (numpy arrays,
keyed as in reference.setup_inputs()) and returns the FULL [B, T, TAGS, TAGS]
crf_scores array.

Sharding: the 512-step scan is latency-bound (per-step serial chain of
~2us across PE->ACT->VEC->ACT->VEC), and per-instruction costs are almost
entirely fixed overhead, so batching all 64 sequences into one op costs
barely more than 8.  We therefore split TIME, not batch: core c computes
time chunk [64c, 64c+64) for the full batch, running its fwd/bwd scans W
extra warmup steps from zero state.  The LSTM here is strongly contractive
(forget gate ~0.5), so the warmup truncation error decays ~0.5^W.

Cell math is reformulated tanh-only (sigma(x) = (tanh(x/2)+1)/2, with
weights pre-scaled on host, hidden state carried as h' = 2h and cell
state as cc = 2c):
    T = tanh(z')            z' blocks [o, i, f, g] with f,i,o halved
    P = [(T_i+1)*T_g | (T_f+1)*cc]      (one scalar_tensor_tensor)
    cc_new = 0.5*P_f + P_i              (one scalar_tensor_tensor,
                                         written into next step's Q tile)
    TC = tanh(0.5*cc_new)               (ACT free scale)
    h' = (T_o+1)*TC                     (one scalar_tensor_tensor)
This drops one VEC op + one ACT op vs the plain sigmoid chain and keeps
the per-step serial path at PE -> ACT -> VEC -> VEC -> ACT -> VEC.
"""
import sys
import types
from contextlib import ExitStack

import ml_dtypes
import numpy as np

import concourse.bacc as bacc
import concourse.bass as bass
import concourse.mybir as mybir
import concourse.tile as tile
from concourse import library_config
from concourse.bass_utils import run_bass_kernel_spmd

# ---- problem dims (hardcoded per spec) ----
VOCAB = 30000
VPAD = 30720      # emb table padded; row 30000 is all-zeros (warmup token)
ZTOK = 30000
EMB = 256
HD = 128          # per-direction hidden
G4 = 512          # 4*HD gates
TAGS = 16
B, T = 64, 512
NCORES = 8

CH = 64           # time chunk per core
W = 16            # warmup steps
S = CH + W        # scan steps per direction
NTS = CH + 2 * W  # timesteps of tokens/zin per core (union fwd+bwd windows)
NTOK = NTS * B    # tokens per core
TPB = 8           # timesteps per 512-token block
NB = NTS // TPB   # gather/gemm blocks
WB = W // TPB     # warmup blocks per end

BF16 = mybir.dt.bfloat16
F32 = mybir.dt.float32
I16 = mybir.dt.int16
AF = mybir.ActivationFunctionType
ALU = mybir.AluOpType

# gate order in reference (jnp.split): i, f, g, o.  Device block order is
# [o, i, f, g] so that W-pair = [T_i|T_f] and V-pair = [T_g|c] are both
# contiguous ascending (c lives in block 4 of the Q tile).
_PERM = np.concatenate([
    np.arange(384, 512),   # o
    np.arange(0, 128),     # i
    np.arange(128, 256),   # f
    np.arange(256, 384),   # g
])
# per-block extra scale for the tanh-half trick: f,i,o rows halved; g not.
_BLK_SCALE = np.concatenate([
    np.full(128, 0.5),     # o
    np.full(128, 0.5),     # i
    np.full(128, 0.5),     # f
    np.full(128, 1.0),     # g
])


def _ensure_ntff_hook():
    """The RL image's antenv lacks axon_hooks; inject it so trace=True works."""
    if "antenv.axon_hooks" in sys.modules:
        return
    mod = types.ModuleType("antenv.axon_hooks")
    mod._hook = None
    mod.set_axon_ntff_profile_hook = lambda h: setattr(mod, "_hook", h)
    mod.get_axon_ntff_profile_hook = lambda: mod._hook
    sys.modules["antenv.axon_hooks"] = mod
    try:
        import antenv
        antenv.axon_hooks = mod
        from trn_agent_boot.trn_boot import _ntff_profile_via_ctypes
        mod.set_axon_ntff_profile_hook(
            _ntff_profile_via_ctypes("/opt/axon/libaxon_pjrt.so"))
    except Exception:
        pass


def build(steps: int = S, nts: int = NTS):
    """Build + compile the per-core Bass program."""
    nb = nts // TPB
    ntok = nts * B
    ch = steps - W
    nc = bacc.Bacc("TRN2", target_bir_lowering=False, debug=False)

    # ---- DRAM I/O ----
    emb_d = nc.dram_tensor("emb", [VPAD, EMB], BF16, kind="ExternalInput")
    idx_d = nc.dram_tensor("idx", [128, ntok // 16], I16, kind="ExternalInput")
    wihT_d = {d: nc.dram_tensor(f"wihT_{d}", [EMB, G4], BF16, kind="ExternalInput")
              for d in "fb"}
    whhT_d = {d: nc.dram_tensor(f"whhT_{d}", [HD, G4], BF16, kind="ExternalInput")
              for d in "fb"}
    # main bias + warmup-window bias (zeroed on edge cores), [128, 4] f32
    bias_d = {d: nc.dram_tensor(f"bias_{d}", [128, 4], F32, kind="ExternalInput")
              for d in "fb"}
    biasw_d = {d: nc.dram_tensor(f"biasw_{d}", [128, 4], F32, kind="ExternalInput")
               for d in "fb"}
    woutT_d = nc.dram_tensor("woutT", [2, HD, TAGS], BF16, kind="ExternalInput")
    trans_d = nc.dram_tensor("trans", [128, TAGS * TAGS], F32, kind="ExternalInput")
    ident_d = nc.dram_tensor("ident", [128, 128], BF16, kind="ExternalInput")
    crf_d = nc.dram_tensor("crf", [ch * B, TAGS * TAGS], F32, kind="ExternalOutput")

    with tile.TileContext(nc) as tc, ExitStack() as ctx:
        nc.gpsimd.load_library(library_config.mlp)
        const = ctx.enter_context(tc.tile_pool(name="const", bufs=1))
        big = ctx.enter_context(tc.tile_pool(name="big", bufs=1))
        # emission pools opened early so their PSUM banks / SBUF slots are
        # disjoint from the scan pools (no pool-release serialization).
        epsum = ctx.enter_context(tc.tile_pool(name="epsum", bufs=2, space="PSUM"))
        ecrf = ctx.enter_context(tc.tile_pool(name="ecrf", bufs=8))

        # ---- persistent SBUF ----
        idx_sb = const.tile([128, ntok // 16], I16)
        wihT = {d: const.tile([128, 2, G4], BF16, tag=f"wihT{d}", name=f"wihT{d}") for d in "fb"}
        whhT = {d: const.tile([HD, G4], BF16, tag=f"whhT{d}", name=f"whhT{d}") for d in "fb"}
        bias = {d: const.tile([128, 4], F32, tag=f"bias{d}", name=f"bias{d}") for d in "fb"}
        biasw = {d: const.tile([128, 4], F32, tag=f"biasw{d}", name=f"biasw{d}") for d in "fb"}
        woutT = const.tile([HD, 2, TAGS], BF16)
        trans = const.tile([128, TAGS * TAGS], F32)
        ident = const.tile([128, 128], BF16)

        # token embeddings, transposed: [128 emb-part, block, emb-half, 512 tok]
        xT = big.tile([128, nb, 2, 512], BF16, tag="xT")
        # quarter-granular staging for the fast-path first blocks (gather
        # out APs must be contiguous, so quarters get their own slots)
        xq = big.tile([128, 16, 2, 128], BF16, tag="xq")
        # input projections, [128 gate-part, t, block(oifg), batch]
        zin = {d: big.tile([128, nts, 4, B], BF16, tag=f"zin{d}", name=f"zin{d}")
               for d in "fb"}
        # h' histories (bf16), split into 16-col segments so the emission
        # epilogue's tile-granular deps bind to a segment (overlaps the
        # scan) instead of the whole history.
        # fwd: col k+1 = h' after fwd step k; real time t0+i at col W+1+i.
        # bwd: step j writes col steps-j (descending); real time t0+i at
        #   col i+1; col steps+1 is the zero init.
        # Segments: [0..W], then 16-col segments covering the real cols.
        hbnd = [0, W + 1] + [W + 1 + 16 * i for i in range(1, ch // 16 + 1)] \
            + [steps + 2]
        hseg = {d: [big.tile([128, hbnd[i + 1] - hbnd[i], B], BF16,
                             tag=f"h{d}{i}", name=f"h{d}{i}")
                    for i in range(len(hbnd) - 1)] for d in "fb"}

        def hcol(d, col):
            for i in range(len(hbnd) - 1):
                if col < hbnd[i + 1]:
                    return hseg[d][i][:, col - hbnd[i], :]
            raise AssertionError(col)

        def hspan2(d, c0):
            """[128, 2, B] span over cols c0, c0+1 (same segment)."""
            for i in range(len(hbnd) - 1):
                if c0 < hbnd[i + 1]:
                    assert c0 + 2 <= hbnd[i + 1], (d, c0)
                    return hseg[d][i][:, c0 - hbnd[i]:c0 - hbnd[i] + 2, :]
            raise AssertionError(c0)

        # ---- load inputs: idx first — it is the only DMA the gathers
        # need, and the SP engine issues DMAs serially at ~650ns each, so
        # the weight/const DMAs are emitted AFTER the gathers below.
        nc.sync.dma_start(idx_sb[:], idx_d[:])
        nc.gpsimd.memset(hcol("f", 0), 0.0)
        nc.gpsimd.memset(hcol("b", steps + 1), 0.0)

        def load_consts():
            for d in "fb":
                nc.sync.dma_start(wihT[d][:], wihT_d[d].rearrange("(k p) g -> p k g", p=128))
                nc.sync.dma_start(whhT[d][:], whhT_d[d][:])
                nc.sync.dma_start(bias[d][:], bias_d[d][:])
                nc.sync.dma_start(biasw[d][:], biasw_d[d][:])
            nc.sync.dma_start(woutT[:], woutT_d.rearrange("c h t -> h c t"))
            nc.sync.dma_start(trans[:], trans_d[:])
            nc.sync.dma_start(ident[:], ident_d[:])

        # ---- embedding gather straight into x.T layout ----
        # fwd consumes blocks ascending from 0, bwd descending from nb-1.
        # The first two blocks of each direction are gathered in 128-token
        # quarters, interleaved by first-use time, so the scan can start
        # ~17us earlier (it only needs f:(0,q0) and b:(nb-1,q3) up front).
        def gather_q(qi, b_, q):
            nc.gpsimd.dma_gather(
                xq[:, qi, :, :],
                emb_d[:, :],
                idx_sb[:, 32 * b_ + 8 * q:32 * b_ + 8 * (q + 1)],
                128, 128, EMB,
                transpose=True,
            )

        quarters = []
        for j in range(8):
            quarters.append(("f", j // 4, j % 4))          # blocks 0,1 fwd
            quarters.append(("b", nb - 1 - j // 4, 3 - j % 4))  # nb-1, nb-2
        border = []
        for k in range(2, (nb + 1) // 2):
            border.append(k)
            if nb - 1 - k != k:
                border.append(nb - 1 - k)

        # ---- input projections: zin = x @ Wih.T + b ----
        # fwd needs blocks [0, nb-1-WB), bwd needs [WB, nb).  Warmup-window
        # blocks (first WB for fwd, last WB for bwd) use the biasw tensor.
        zpsum = ctx.enter_context(tc.tile_pool(name="zpsum", bufs=2, space="PSUM"))
        _flip = [0]

        def zin_unit(d, b_, c):
            """One (dir, block, gate-slice) GEMM + bias copyback."""
            warm = (d == "f" and b_ < WB) or (d == "b" and b_ >= nb - WB)
            bsel = biasw[d] if warm else bias[d]
            zp = zpsum.tile([128, 512], F32, tag="zp")
            nc.tensor.matmul(
                zp[:], wihT[d][:, 0, 128 * c:128 * (c + 1)],
                xT[:, b_, 0, :],
                start=True, stop=False)
            nc.tensor.matmul(
                zp[:], wihT[d][:, 1, 128 * c:128 * (c + 1)],
                xT[:, b_, 1, :],
                start=False, stop=True)
            # strided copyback into [t, c, b] layout, bias folded in;
            # alternate ACT/VEC to balance engine load.
            dst = zin[d][:, TPB * b_:TPB * (b_ + 1), c, :]
            if _flip[0] % 2 == 0:
                nc.scalar.activation(dst, zp[:], AF.Identity,
                                     bias=bsel[:, c:c + 1])
            else:
                nc.vector.tensor_scalar(dst, zp[:], bsel[:, c:c + 1],
                                        None, ALU.add)
            _flip[0] += 1

        def zin_unit_q(qi, d, b_, c, q):
            """Quarter-block (128-token) GEMM + bias copyback."""
            warm = (d == "f" and b_ < WB) or (d == "b" and b_ >= nb - WB)
            bsel = biasw[d] if warm else bias[d]
            zq = zpsum.tile([128, 128], F32, tag="zp")
            nc.tensor.matmul(
                zq[:], wihT[d][:, 0, 128 * c:128 * (c + 1)],
                xq[:, qi, 0, :],
                start=True, stop=False)
            nc.tensor.matmul(
                zq[:], wihT[d][:, 1, 128 * c:128 * (c + 1)],
                xq[:, qi, 1, :],
                start=False, stop=True)
            dst = zin[d][:, TPB * b_ + 2 * q:TPB * b_ + 2 * q + 2, c, :]
            if _flip[0] % 2 == 0:
                nc.scalar.activation(dst, zq[:], AF.Identity,
                                     bias=bsel[:, c:c + 1])
            else:
                nc.vector.tensor_scalar(dst, zq[:], bsel[:, c:c + 1],
                                        None, ALU.add)
            _flip[0] += 1

        # Up-front fast path: quarter-granular gathers + GEMMs for the
        # first two blocks of each direction, interleaved by need time.
        # The rest is emitted inside the scan loop (one unit per step) so
        # the zin GEMMs fill engine-idle slots instead of fighting the
        # chain at scan start.
        for qi, (d, b_, q) in enumerate(quarters):
            gather_q(qi, b_, q)
        load_consts()
        for qi, (d, b_, q) in enumerate(quarters):
            for c in range(4):
                zin_unit_q(qi, d, b_, c, q)
        for b_ in border:
            nc.gpsimd.dma_gather(
                xT[:, b_, :, :],
                emb_d[:, :],
                idx_sb[:, 32 * b_:32 * (b_ + 1)],
                512, 512, EMB,
                transpose=True,
            )

        # Deferred-block zin units: upfront emission measured best — the
        # copyback intrusions concentrate in the first ~20 scan steps and
        # the rest of the scan runs at the clean 2213ns chain period.
        # (Spreading them through the loop locked a +437ns/step regime.)
        for j in range(2, nb - 2):
            for d, b_ in (("f", j), ("b", nb - 1 - j)):
                for c in range(4):
                    zin_unit(d, b_, c)

        def inloop_units(k):
            return []

        # ---- the recurrent scan (fwd + bwd interleaved) ----
        with tc.tile_pool(name="spsum", bufs=4, space="PSUM") as spsum, \
             tc.tile_pool(name="sQ", bufs=10) as sQ, \
             tc.tile_pool(name="sP", bufs=10) as sP, \
             tc.tile_pool(name="sT", bufs=8) as sT:

            def new_z(k):
                """Fresh psum tiles for step k with zin injected (identity mm).
                Emitted one step ahead so gate mms fire as soon as h lands."""
                zt = {}
                for d in "fb":
                    ti = k if d == "f" else nts - 1 - k
                    zt[d] = spsum.tile([128, 4, B], F32, tag="z", name=f"z{d}")
                    nc.tensor.matmul(zt[d][:], ident[:],
                                     zin[d][:, ti, :, :],
                                     start=True, stop=False)
                return zt

            q = {d: sQ.tile([128, 5, B], F32, tag="q", name=f"q{d}") for d in "fb"}
            for d in "fb":
                nc.gpsimd.memset(q[d][:, 4, :], 0.0)
            z = new_z(0)
            for k in range(steps):
                for d, rd_col in (("f", k), ("b", steps + 1 - k)):
                    for c in range(4):
                        nc.tensor.matmul(
                            z[d][:, c, :],
                            whhT[d][:, 128 * c:128 * (c + 1)],
                            hcol(d, rd_col),
                            start=False, stop=(c == 3))
                z_cur, z = z, (new_z(k + 1) if k + 1 < steps else None)
                qn = {d: sQ.tile([128, 5, B], F32, tag="q", name=f"q{d}")
                      for d in "fb"}
                # Stage-interleaved emission: both dirs' ops alternate at
                # each chain stage so the engine queues lock the two chains
                # half a period out of phase.
                wr_col = {"f": k + 1, "b": steps - k}
                P = {}
                for d in "fb":
                    # T = tanh(z') into blocks [o,i,f,g]; state cc = 2*c
                    # sits in block 4.
                    nc.scalar.activation(q[d][:, 0:4, :], z_cur[d][:],
                                         AF.Tanh)
                for d in "fb":
                    # P = [(T_i+1)*T_g | (T_f+1)*cc]
                    P[d] = sP.tile([128, 2, B], F32, tag="P", name="P")
                    nc.vector.scalar_tensor_tensor(
                        P[d][:], q[d][:, 1:3, :], 1.0, q[d][:, 3:5, :],
                        ALU.add, ALU.mult)
                for d in "fb":
                    # cc_new = 2*c_new = 0.5*P1 + P0, written straight into
                    # the next step's Q tile (no separate state-fix op).
                    nc.vector.scalar_tensor_tensor(
                        qn[d][:, 4, :], P[d][:, 1, :], 0.5, P[d][:, 0, :],
                        ALU.mult, ALU.add)
                TC = {}
                for d in "fb":
                    TC[d] = sT.tile([128, B], F32, tag="TC", name="TC")
                    nc.scalar.activation(TC[d][:], qn[d][:, 4, :], AF.Tanh,
                                         scale=0.5)
                for d in "fb":
                    # h' = (T_o + 1) * TC
                    nc.vector.scalar_tensor_tensor(
                        hcol(d, wr_col[d]), q[d][:, 0, :], 1.0, TC[d][:],
                        ALU.add, ALU.mult)
                q = qn
                for (ud, ub, uc) in inloop_units(k):
                    zin_unit(ud, ub, uc)

        # ---- emission + CRF broadcast-add + store ----
        # chunk n covers local times 2n, 2n+1 (128 tokens);
        # hf cols W+1+2n..W+2+2n, hb cols 2n+1..2n+2.
        nchunks = ch // 2
        order = sorted(range(nchunks),
                       key=lambda n: max(W + 2 + 2 * n, steps - 1 - 2 * n))
        if True:
            for n in order:
                e = epsum.tile([128, TAGS], F32, tag="e")
                nc.tensor.matmul(e[:], hspan2("f", W + 1 + 2 * n),
                                 woutT[:, 0, :], start=True, stop=False)
                nc.tensor.matmul(e[:], hspan2("b", 1 + 2 * n),
                                 woutT[:, 1, :], start=False, stop=True)
                crf_sb = ecrf.tile([128, TAGS * TAGS], F32, tag="crf")
                e_b = e[:, None, :].to_broadcast([128, TAGS, TAGS])
                nc.vector.tensor_tensor(crf_sb[:], e_b, trans[:], ALU.add)
                nc.sync.dma_start(crf_d[128 * n:128 * (n + 1), :], crf_sb[:])

    nc.compile()
    _assert_ldw_pairing(nc)
    return nc


def _assert_ldw_pairing(nc):
    """Every non-self-loading matmul must directly follow an InstLdweights
    whose weights AP matches the matmul's weights operand."""
    for f in nc.m.functions:
        for bb in f.blocks:
            prev_pe = None
            for ins in bb.instructions:
                if ins.engine != mybir.EngineType.PE:
                    continue
                if isinstance(ins, mybir.InstMatmult) and ins.ldweights is False:
                    assert isinstance(prev_pe, mybir.InstLdweights), (
                        f"{ins.name}: non-self-loading matmul not preceded by "
                        f"ldweights (got {type(prev_pe).__name__})")
                    assert repr(prev_pe.ins[0]) == repr(ins.ins[1]), (
                        f"{ins.name}: weights mismatch with {prev_pe.name}")
                prev_pe = ins


_CACHE = {}


def _get_nc():
    if "nc" not in _CACHE:
        _CACHE["nc"] = build()
    return _CACHE["nc"]


def _prep_dir(w_ih, w_hh, b):
    """Permute gates to [o,i,f,g]; apply tanh-half trick (f,i,o rows x0.5)
    and h'=2h compensation (all Whh x0.5)."""
    w_ih = np.asarray(w_ih, np.float32)[_PERM] * _BLK_SCALE[:, None]
    w_hh = np.asarray(w_hh, np.float32)[_PERM] * (0.5 * _BLK_SCALE[:, None])
    b = np.asarray(b, np.float32)[_PERM] * _BLK_SCALE
    wihT = np.ascontiguousarray(w_ih.T).astype(ml_dtypes.bfloat16)
    whhT = np.ascontiguousarray(w_hh.T).astype(ml_dtypes.bfloat16)
    bias = np.ascontiguousarray(b.reshape(4, 128).T).astype(np.float32)
    return wihT, whhT, bias


def make_in_maps(sentences, embedding, W_ih_f, W_hh_f, b_f, W_ih_b, W_hh_b,
                 b_b, W_out, b_out, transition):
    emb = np.zeros((VPAD, EMB), np.float32)
    emb[:VOCAB] = np.asarray(embedding, np.float32)
    emb = emb.astype(ml_dtypes.bfloat16)
    wihT_f, whhT_f, bias_f = _prep_dir(W_ih_f, W_hh_f, b_f)
    wihT_b, whhT_b, bias_b = _prep_dir(W_ih_b, W_hh_b, b_b)
    wo = np.asarray(W_out, np.float32) * 0.5   # h' = 2h compensation
    woutT = np.stack([np.ascontiguousarray(wo[:, :128].T),
                      np.ascontiguousarray(wo[:, 128:].T)])
    woutT = woutT.astype(ml_dtypes.bfloat16)  # [2, 128, 16]
    trans_aug = (np.asarray(transition, np.float32)
                 + np.asarray(b_out, np.float32)[None, :]).reshape(-1)  # [256]
    trans_rep = np.ascontiguousarray(
        np.broadcast_to(trans_aug, (128, 256))).astype(np.float32)
    ident = np.eye(128, dtype=ml_dtypes.bfloat16)
    zeros4 = np.zeros((128, 4), np.float32)

    # tokens per core: times [64c - W, 64c + 64 + W), batch-inner (t, b)
    # order; out-of-range times -> the zero embedding row (ZTOK).
    sent = np.asarray(sentences).astype(np.int64)  # [B, T]
    in_maps = []
    for c in range(NCORES):
        t_lo = CH * c - W
        times = np.arange(t_lo, t_lo + NTS)
        cols = np.clip(times, 0, T - 1)
        toks = sent[:, cols].T.copy()          # [NTS, B]
        toks[(times < 0) | (times >= T)] = ZTOK
        toks = toks.reshape(-1)                # (t, b) order, [NTOK]
        idx = np.tile(toks.reshape(NTOK // 16, 16).T.astype(np.int16), (8, 1))
        in_maps.append({
            "emb": emb, "idx": idx,
            "wihT_f": wihT_f, "wihT_b": wihT_b,
            "whhT_f": whhT_f, "whhT_b": whhT_b,
            "bias_f": bias_f, "bias_b": bias_b,
            "biasw_f": zeros4 if c == 0 else bias_f,
            "biasw_b": zeros4 if c == NCORES - 1 else bias_b,
            "woutT": woutT, "trans": trans_rep, "ident": ident,
        })
    return in_maps


def assemble_out(results):
    out = np.empty((B, T, TAGS, TAGS), np.float32)
    for c in range(NCORES):
        crf = results[c]["crf"].reshape(CH, B, TAGS, TAGS)
        out[:, CH * c:CH * (c + 1)] = crf.transpose(1, 0, 2, 3)
    return out


def kernel(**inputs):
    _ensure_ntff_hook()
    nc = _get_nc()
    in_maps = make_in_maps(**inputs)
    res = run_bass_kernel_spmd(nc, in_maps, list(range(NCORES)))
    return assemble_out(res.results)
